# revision 1
# baseline (speedup 1.0000x reference)
"""Transformer decoder layer (pre-norm, self-attn + cross-attn + FFN) on 8
Trainium2 NeuronCores.

Sharding: core c handles batch b = c//2 and the contiguous half of the 1024
target tokens h = c%2 (512 query rows each). K/V work is duplicated within
each batch pair so there are no collectives; every core runs an identical
program on different data. The host rotates the token order per core so that
each core's own tokens are always columns [0, 512) -> one uniform SPMD
program.

On-device layout is feature-major ([d_model, token]) throughout, so no
on-device transposes are needed: the host pre-transposes x / memory / weights
and transposes the output back. Masks are converted to additive fp32 masks on
the host. Matmuls run in float32r (full PE rate at free dim 512).
"""

import numpy as np
from contextlib import ExitStack

import concourse.bass as bass
import concourse.bacc as bacc
import concourse.tile as tile
from concourse import mybir
from concourse.bass_utils import run_bass_kernel_spmd

D = 1024        # d_model
H = 16          # heads
DK = 64         # head dim
DFF = 4096
B = 4
T = 1024        # tgt/src len
OWN = 512       # query rows per core
P = 128         # partitions
NKC = D // P    # 8 feature chunks
NSC = T // P    # 8 s-chunks
NFC = DFF // P  # 32 ffn chunks
EPS = 1e-6

F32 = mybir.dt.float32
F32R = mybir.dt.float32r
AF = mybir.ActivationFunctionType
ALU = mybir.AluOpType


def _r(ap):
    return ap.bitcast(F32R)


# ---------------------------------------------------------------------------
# program builder (identical for every core; only DRAM contents differ)
# ---------------------------------------------------------------------------

def build_program(repeat=1, mask_mode="split"):
    nc = bacc.Bacc(None)
    dr = {}

    def din(name, shape, dt=F32):
        dr[name] = nc.dram_tensor(name, list(shape), dt, kind="ExternalInput")
        return dr[name]

    din("ones_r", [P, P], F32R)
    din("xT", [D, T], F32R)                # batch-b x, transposed, own tokens first
    din("memT", [D, T], F32R)              # memory[b] transposed
    din("maskT", [T, OWN])           # additive tgt mask, [s_rot, q_local]
    din("lnwrows", [1, 3 * D], F32R)  # LN w rows, concat (PE bcast)
    # all small per-partition tensors packed into one DMA:
    # cols: [sa_bq 8][sa_bk 8][sa_bo 8][ca_bq 8][ca_bk 8][ca_bo 8]
    #       [b1 32][b2 8][ln1_b 8][ln2_b 8][ln3_b 8][smask 8][tailb 4]
    din("smalls", [P, 124])
    for pre in ("sa", "ca"):
        din(f"{pre}_wq", [NKC, P, D], F32R)
        din(f"{pre}_wk", [NKC, P, D], F32R)
        din(f"{pre}_wv", [NKC, P, D], F32R)  # V weights: W^T row-chunks
        din(f"{pre}_wo", [NKC, P, D], F32R)
        din(f"{pre}_bv", [D], F32R)      # placed at partition 64 (outer trick)
    din("w1", [NKC, P, DFF], F32R)
    din("w2", [NFC, P, D], F32R)

    outT = nc.dram_tensor("outT", [D, OWN], F32, kind="ExternalOutput")

    with ExitStack() as ctx:
        tc = ctx.enter_context(tile.TileContext(nc))
        ctx.enter_context(nc.allow_low_precision(
            reason="float32r staging for full-rate PE matmuls"))
        persist = ctx.enter_context(tc.tile_pool(name="persist", bufs=1))

        ones = persist.tile([P, P], F32R, tag="ones", name="ones")
        nc.gpsimd.dma_start(ones[:], dr["ones_r"][:])
        ones_f = persist.tile([P, 1], F32, tag="ones_f", name="ones_f")
        nc.vector.memset(ones_f[:], 1.0)
        eps_11 = persist.tile([1, 1], F32, tag="eps11", name="eps11")
        nc.vector.memset(eps_11[:], EPS)

        smalls = persist.tile([P, 124], F32, tag="smalls", name="smalls")
        nc.gpsimd.dma_start(smalls[:], dr["smalls"][:])
        _off = [0]

        def s_col(n):
            t = smalls[:, _off[0]:_off[0] + n]
            _off[0] += n
            return t

        bias = {}
        for pre in ("sa", "ca"):
            for nm in ("bq", "bk", "bo"):
                bias[f"{pre}_{nm}"] = s_col(NKC)
        bias["b1"] = s_col(NFC)
        bias["b2"] = s_col(NKC)
        lnp = {}
        for ln in ("ln1", "ln2", "ln3"):
            lnp[f"{ln}_b"] = s_col(NKC)
        smask = s_col(NSC)
        tailb = s_col(4)
        for pre in ("sa", "ca"):
            bv = persist.tile([P, D], F32R, tag=f"{pre}_bv", name=f"{pre}_bv")
            nc.gpsimd.dma_start(bv[64:65, :], dr[f"{pre}_bv"][None, :])
            bias[f"{pre}_bv"] = bv
        lnwr = persist.tile([1, 3 * D], F32R, tag="lnwrows", name="lnwrows")
        nc.gpsimd.dma_start(lnwr[:], dr["lnwrows"][:])
        lnrow = {"ln1": lnwr[0:1, 0:D], "ln2": lnwr[0:1, D:2 * D],
                 "ln3": lnwr[0:1, 2 * D:3 * D]}

        # residual (own tokens), lives to the end
        xcur = [persist.tile([P, OWN], F32, tag=f"xc{i}", name=f"xc{i}")
                for i in range(NKC)]

        tmp = ctx.enter_context(tc.tile_pool(name="tmp", bufs=2))
        stats = ctx.enter_context(tc.tile_pool(name="stats", bufs=1))

        # ------------------------------------------------------------------
        def layer_norm(nblk, src_get, dst_blocks, wrow, b_pn, ps_st):
            """Feature-major LN, h = (x - mean) * (w * rstd) + b.
            src_get(blk, kc) returns a [P, 512] AP; it may stream a fresh
            tile per call (it is called twice per chunk). w is folded into
            the PE broadcast (outer product w (x) rstd): 2-pass DVE
            normalize."""
            for blk in range(nblk):
                db = dst_blocks[blk]
                sb0 = src_get(blk, 0)
                src_is_r = sb0.dtype == F32R
                ones_s = ones[:, 0:1] if src_is_r else ones_f[:]
                ps_s = ps_st.tile([1, 512], F32, tag="lns", name="lns")
                ps_q = ps_st.tile([1, 512], F32, tag="lnq", name="lnq")
                for kc in range(NKC):
                    sbk = sb0 if kc == 0 else src_get(blk, kc)
                    nc.tensor.matmul(ps_s[:], ones_s, sbk,
                                     start=(kc == 0), stop=(kc == NKC - 1))
                    sq = tmp.tile([P, 512], F32R, tag="lnsq", name="lnsq", bufs=1)
                    nc.scalar.activation(sq[:], sbk, AF.Square)
                    nc.tensor.matmul(ps_q[:], ones[:, 0:1], sq[:],
                                     start=(kc == 0), stop=(kc == NKC - 1))
                s2 = stats.tile([1, 512], F32, tag="lnstA", name="lnstA")
                # s2 = (sum/sqrt(D*(D-1)))^2 = sum^2/(D*(D-1))
                nc.scalar.activation(s2[:], ps_s[:], AF.Square,
                                     scale=float(1.0 / np.sqrt(D * (D - 1.0))))
                var = stats.tile([1, 512], F32, tag="lnstB", name="lnstB")
                nc.vector.scalar_tensor_tensor(
                    var[:], ps_q[:], 1.0 / (D - 1.0), s2[:],
                    op0=ALU.mult, op1=ALU.subtract)
                std = stats.tile([1, 512], F32, tag="lnstA", name="lnstA")
                nc.scalar.activation(std[:], var[:], AF.Sqrt)
                nc.scalar.add(std[:], std[:], eps_11[:])
                rstd = stats.tile([1, 512], F32R, tag="lnstC",
                                  name="lnstC")
                nc.vector.reciprocal(rstd[:], std[:])
                mr = stats.tile([1, 512], F32R, tag="lnstB", name="lnstB")
                nc.vector.scalar_tensor_tensor(
                    mr[:], ps_s[:], 1.0 / D, rstd[:],
                    op0=ALU.mult, op1=ALU.mult)
                for kc in range(NKC):
                    wl = wrow[0:1, kc * P:(kc + 1) * P]
                    ps_rb = ps_st.tile([P, 512], F32, tag="ln_rb",
                                       name="ln_rb", bufs=1)
                    nc.tensor.matmul(ps_rb[:], wl, rstd[:],
                                     start=True, stop=True)
                    ps_mb = ps_st.tile([P, 512], F32, tag="ln_mb",
                                       name="ln_mb", bufs=1)
                    nc.tensor.matmul(ps_mb[:], wl, mr[:],
                                     start=True, stop=True)
                    t = tmp.tile([P, 512], F32, tag="lnt", name="lnt")
                    nc.vector.tensor_mul(t[:], src_get(blk, kc), ps_rb[:])
                    nc.vector.scalar_tensor_tensor(
                        db[kc], t[:], b_pn[:, kc:kc + 1], ps_mb[:],
                        op0=ALU.add, op1=ALU.subtract)

        # ------------------------------------------------------------------
        def load_w_rows(wpool, wname, n=NKC, cols=D):
            tiles = []
            for kc in range(n):
                wt = wpool.tile([P, cols], F32R, tag="w", name="w")
                nc.sync.dma_start(wt[:], dr[wname][kc])
                tiles.append(wt)
            return tiles

        def proj_fm(wpool, wname, bias_pn, src_blocks, dst, ps_acc):
            """dst[c][:, blk*512:..] = sum_kc W^T[kc,c].T @ src[blk][kc] + b.
            Full W^T resident as 8 [P, D] row-chunk tiles (one DMA each)."""
            nblk = len(src_blocks)
            w_tiles = load_w_rows(wpool, wname)
            for c in range(NKC):
                pss = [ps_acc.tile([P, 512], F32, tag=f"proj{blk}",
                                   name=f"proj{blk}") for blk in range(nblk)]
                for kc in range(NKC):
                    for blk in range(nblk):
                        nc.tensor.matmul(pss[blk][:],
                                         w_tiles[kc][:, c * P:(c + 1) * P],
                                         src_blocks[blk][kc],
                                         start=(kc == 0), stop=(kc == NKC - 1))
                for blk in range(nblk):
                    nc.vector.tensor_scalar_add(
                        dst[c][:, blk * 512:(blk + 1) * 512],
                        pss[blk][:], bias_pn[:, c:c + 1])

        def proj_tm_vaug(wpool, wname, src_blocks, vaug, ps_acc):
            """Token-major V projection into [P, H, DK+1] aug tiles."""
            wv = load_w_rows(wpool, wname)
            for dc in range(2):
                for st in range(NSC):
                    sb = src_blocks[st // 4]
                    t0 = (st % 4) * P
                    ps = ps_acc.tile([P, 512], F32, tag="proj0", name="proj0")
                    for kc in range(NKC):
                        nc.tensor.matmul(
                            ps[:], sb[kc][:, t0:t0 + P],
                            wv[kc][:, dc * 512:(dc + 1) * 512],
                            start=(kc == 0), stop=(kc == NKC - 1))
                    nc.vector.tensor_copy(
                        vaug[st][:, 8 * dc:8 * (dc + 1), 0:DK],
                        ps[:].rearrange("p (h d) -> p h d", h=8))

        # ------------------------------------------------------------------
        def attention(KT, QT, vaug, OT, mask_tiles, tail_pn, smask_pn,
                      bv_tile, att_pools):
            ps_sc, ps_av, ps_rb, epool = att_pools
            nmask = len(mask_tiles) if mask_tiles is not None else 0
            for c in range(NKC):
                po = [ps_av.tile([DK + 1, 512], F32, tag="po0", name="po0"),
                      ps_av.tile([DK + 1, 512], F32, tag="po1", name="po1")]
                for i in range(NSC):
                    pssc = [ps_sc.tile([P, 512], F32, tag="sc", name="sc"),
                            ps_sc.tile([P, 512], F32, tag="sc", name="sc")]
                    for h01 in (0, 1):
                        sl = slice(64 * h01, 64 * h01 + 64)
                        nc.tensor.matmul(
                            pssc[h01][:], KT[c][sl, i * P:(i + 1) * P],
                            QT[c][sl, :], start=True, stop=True)
                    for h01 in (0, 1):
                        e = epool.tile([P, 512], F32R, tag="e", name="e")
                        if i < nmask:
                            tm = epool.tile([P, 512], F32, tag="emask",
                                            name="emask", bufs=2)
                            nc.vector.tensor_add(tm[:], pssc[h01][:],
                                                 mask_tiles[i][:])
                            nc.scalar.activation(e[:], tm[:], AF.Exp)
                        elif tail_pn is not None:
                            nc.scalar.activation(
                                e[:], pssc[h01][:], AF.Exp,
                                bias=tail_pn[:, i - 4:i - 3])
                        elif smask_pn is not None:
                            nc.scalar.activation(e[:], pssc[h01][:], AF.Exp,
                                                 bias=smask_pn[:, i:i + 1])
                        else:
                            nc.scalar.activation(e[:], pssc[h01][:], AF.Exp)
                        nc.tensor.matmul(
                            po[h01][:], vaug[i][:, 2 * c + h01, :],
                            e[:], start=(i == 0), stop=(i == NSC - 1))
                for h01 in (0, 1):
                    h = 2 * c + h01
                    sums = epool.tile([P, 512], F32R, tag="sums", name="sums", bufs=2)
                    nc.vector.tensor_copy(sums[64:65, :],
                                          po[h01][DK:DK + 1, :])
                    # O_un += bv (x) sums  (folds the V bias through softmax)
                    nc.tensor.matmul(
                        po[h01][0:DK, :],
                        bv_tile[64:65, DK * h:DK * h + DK],
                        sums[64:65, :], start=False, stop=True,
                        skip_group_check=True)
                    nc.vector.reciprocal(sums[64:65, :], sums[64:65, :])
                    prb = ps_rb.tile([DK, 512], F32, tag="rb", name="rb")
                    nc.tensor.matmul(prb[:], ones[64:65, 0:DK],
                                     sums[64:65, :], start=True, stop=True)
                    # DVE can read only one PSUM operand: stage po first
                    o_un = epool.tile([DK, 512], F32, tag="o_un", name="o_un", bufs=2)
                    nc.vector.tensor_copy(o_un[:], po[h01][0:DK, :])
                    if h01 == 0:
                        nc.vector.tensor_mul(OT[c][0:DK, :], o_un[:], prb[:])
                    else:
                        ot = epool.tile([DK, 512], F32R, tag="ot", name="ot", bufs=1)
                        nc.vector.tensor_mul(ot[:], o_un[:], prb[:])
                        nc.sync.dma_start(OT[c][DK:P, :], ot[:])

        def out_proj_residual(w_tiles, bias_pn, OT, ps_acc):
            for c2 in range(NKC):
                ps = ps_acc.tile([P, 512], F32, tag="proj0", name="proj0")
                for c in range(NKC):
                    nc.tensor.matmul(ps[:],
                                     w_tiles[c][:, c2 * P:(c2 + 1) * P],
                                     OT[c][:],
                                     start=(c == 0), stop=(c == NKC - 1))
                nc.vector.scalar_tensor_tensor(
                    xcur[c2][:], ps[:], bias_pn[:, c2:c2 + 1], xcur[c2][:],
                    op0=ALU.add, op1=ALU.add)

        def attention_block(prefix, pn, KT, QT, vaug, mask_tiles, tail_pn,
                            use_smask, scope):
            """Runs attention + output projection + residual."""
            otp = scope.enter_context(
                tc.tile_pool(name=f"{pn}_otp", bufs=1))
            OT = [otp.tile([P, OWN], F32R, tag=f"OT{c}", name=f"OT{c}")[:]
                  for c in range(NKC)]
            # wo prefetch overlaps the attention phase
            wsp = scope.enter_context(tc.tile_pool(name=f"{pn}_wso", bufs=8))
            wo_tiles = load_w_rows(wsp, f"{prefix}_wo")
            with ExitStack() as att:
                ps_sc = att.enter_context(
                    tc.tile_pool(name=f"{pn}_psc", bufs=3, space="PSUM"))
                ps_av = att.enter_context(
                    tc.tile_pool(name=f"{pn}_pav", bufs=2, space="PSUM"))
                ps_rb = att.enter_context(
                    tc.tile_pool(name=f"{pn}_prb", bufs=1, space="PSUM"))
                epool = att.enter_context(
                    tc.tile_pool(name=f"{pn}_ep", bufs=3))
                attention(KT, QT, vaug, OT, mask_tiles, tail_pn,
                          smask if use_smask else None,
                          bias[f"{prefix}_bv"], (ps_sc, ps_av, ps_rb, epool))
            with tc.tile_pool(name=f"{pn}_pso", bufs=4, space="PSUM") as pso:
                out_proj_residual(wo_tiles, bias[f"{prefix}_bo"], OT, pso)

        for _rep in range(repeat):
            # ==================================================================
            # phase 1+2: LN1, self-attention
            # ==================================================================
            with ExitStack() as sa:
                big = sa.enter_context(tc.tile_pool(name=f"sa_big{_rep}", bufs=1))
                QT = [big.tile([P, OWN], F32R, tag=f"QT{c}", name=f"QT{c}")[:]
                      for c in range(NKC)]
                KT = [big.tile([P, T], F32R, tag=f"KT{c}", name=f"KT{c}")[:]
                      for c in range(NKC)]
                vaug = [big.tile([P, H, DK + 1], F32R, tag=f"V{i}", name=f"V{i}")[:]
                        for i in range(NSC)]
                for i in range(NSC):
                    nc.sync.dma_start(vaug[i][:, :, DK], dr["ones_r"][:, 0:H])

                with ExitStack() as ph:
                    h1p = ph.enter_context(
                        tc.tile_pool(name=f"h1{_rep}", bufs=1))
                    h1_blocks = [
                        [h1p.tile([P, 512], F32R, tag=f"h1_{blk}_{kc}",
                                  name=f"h1_{blk}_{kc}")[:]
                         for kc in range(NKC)]
                        for blk in range(2)]
                    with ExitStack() as wscope:
                        # weight pool open during LN1 so the Q/K/V weight
                        # prefetch overlaps the LN chain
                        wsp = wscope.enter_context(
                            tc.tile_pool(name=f"sa_ws{_rep}", bufs=9))
                        psa = wscope.enter_context(
                            tc.tile_pool(name=f"ps_sap{_rep}", bufs=2,
                                         space="PSUM"))
                        with ExitStack() as lnscope:
                            xsp = lnscope.enter_context(
                                tc.tile_pool(name=f"xs{_rep}", bufs=3))
                            ps_ln = lnscope.enter_context(
                                tc.tile_pool(name=f"ps_ln1{_rep}", bufs=1,
                                             space="PSUM"))

                            def x_get(blk, kc):
                                xt = xsp.tile([P, 512], F32R, tag="xs",
                                              name="xs")
                                nc.sync.dma_start(
                                    xt[:],
                                    dr["xT"][kc * P:(kc + 1) * P,
                                             blk * 512:(blk + 1) * 512])
                                return xt[:]

                            layer_norm(2, x_get, h1_blocks, lnrow["ln1"],
                                       lnp["ln1_b"], ps_ln)
                        proj_fm(wsp, "sa_wq", bias["sa_bq"], [h1_blocks[0]],
                                QT, psa)
                        proj_fm(wsp, "sa_wk", bias["sa_bk"], h1_blocks,
                                KT, psa)
                        proj_tm_vaug(wsp, "sa_wv", h1_blocks, vaug, psa)

                # residual base (own half of x)
                for kc in range(NKC):
                    nc.sync.dma_start(
                        xcur[kc][:],
                        dr["xT"][kc * P:(kc + 1) * P, 0:OWN].bitcast(F32))

                with ExitStack() as mscope:
                    mp = mscope.enter_context(
                        tc.tile_pool(name=f"maskp{_rep}", bufs=1))
                    nmt = 4 if mask_mode == "split" else NSC
                    mask_tiles = []
                    for i in range(nmt):
                        mt = mp.tile([P, OWN], F32, tag=f"mask{i}",
                                     name=f"mask{i}")
                        nc.sync.dma_start(mt[:],
                                          dr["maskT"][i * P:(i + 1) * P, :])
                        mask_tiles.append(mt[:])
                    attention_block(
                        "sa", f"sa{_rep}", KT, QT, vaug, mask_tiles,
                        tailb if mask_mode == "split" else None,
                        False, mscope)

            # ==================================================================
            # phase 3: cross attention
            # ==================================================================
            with ExitStack() as ca:
                big = ca.enter_context(tc.tile_pool(name=f"ca_big{_rep}", bufs=1))
                QT = [big.tile([P, OWN], F32R, tag=f"cQT{c}", name=f"cQT{c}")[:]
                      for c in range(NKC)]
                KT = [big.tile([P, T], F32R, tag=f"cKT{c}", name=f"cKT{c}")[:]
                      for c in range(NKC)]
                vaug = [big.tile([P, H, DK + 1], F32R, tag=f"cV{i}",
                                 name=f"cV{i}")[:] for i in range(NSC)]
                for i in range(NSC):
                    nc.sync.dma_start(vaug[i][:, :, DK], dr["ones_r"][:, 0:H])

                # K/V projections first: they depend only on memory, so
                # they keep the PE busy while the LN2 chain runs.
                with ExitStack() as mm_scope:
                    wsp = mm_scope.enter_context(
                        tc.tile_pool(name=f"ca_wkv{_rep}", bufs=9))
                    pca = mm_scope.enter_context(
                        tc.tile_pool(name=f"ps_ckv{_rep}", bufs=2,
                                     space="PSUM"))
                    with ExitStack() as mscope2:
                        memp = mscope2.enter_context(
                            tc.tile_pool(name=f"mem{_rep}", bufs=1))
                        mem_blocks = []
                        for blk in range(2):
                            mb = []
                            for kc in range(NKC):
                                mt = memp.tile([P, 512], F32R,
                                               tag=f"m{blk}_{kc}",
                                               name=f"m{blk}_{kc}")
                                nc.sync.dma_start(
                                    mt[:],
                                    dr["memT"][kc * P:(kc + 1) * P,
                                               blk * 512:(blk + 1) * 512])
                                mb.append(mt[:])
                            mem_blocks.append(mb)
                        proj_fm(wsp, "ca_wk", bias["ca_bk"], mem_blocks,
                                KT, pca)
                        proj_tm_vaug(wsp, "ca_wv", mem_blocks, vaug, pca)

                    with ExitStack() as ph:
                        h2p = ph.enter_context(
                            tc.tile_pool(name=f"h2{_rep}", bufs=1))
                        h2 = [h2p.tile([P, OWN], F32R, tag=f"h2_{kc}",
                                       name=f"h2_{kc}")[:]
                              for kc in range(NKC)]
                        with tc.tile_pool(name=f"ps_ln2{_rep}", bufs=1,
                                          space="PSUM") as psl:
                            layer_norm(1, lambda blk, kc: xcur[kc][:], [h2],
                                       lnrow["ln2"], lnp["ln2_b"], psl)
                        proj_fm(wsp, "ca_wq", bias["ca_bq"], [h2], QT, pca)

                with ExitStack() as ascope:
                    attention_block("ca", f"ca{_rep}", KT, QT, vaug, None,
                                None, True, ascope)

            # ==================================================================
            # phase 4: FFN
            # ==================================================================
            with ExitStack() as ff:
                ap_pool = ff.enter_context(tc.tile_pool(name=f"aT{_rep}", bufs=1))
                w2p = ff.enter_context(tc.tile_pool(name=f"w2p{_rep}", bufs=6))
                aT = [ap_pool.tile([P, OWN], F32R, tag=f"aT{i}", name=f"aT{i}")[:]
                      for i in range(NFC)]
                with ExitStack() as ph:
                    h3p = ph.enter_context(
                        tc.tile_pool(name=f"h3{_rep}", bufs=1))
                    h3 = [h3p.tile([P, OWN], F32R, tag=f"h3_{kc}",
                                   name=f"h3_{kc}")[:] for kc in range(NKC)]
                    with ExitStack() as wscope:
                        wsp = wscope.enter_context(
                            tc.tile_pool(name=f"ff_ws{_rep}", bufs=10))
                        ps_f1 = wscope.enter_context(
                            tc.tile_pool(name=f"ps_ff1{_rep}", bufs=4,
                                         space="PSUM"))
                        with tc.tile_pool(name=f"ps_ln3{_rep}", bufs=1,
                                          space="PSUM") as psl:
                            layer_norm(1, lambda blk, kc: xcur[kc][:], [h3],
                                       lnrow["ln3"], lnp["ln3_b"], psl)
                        for cg in range(4):
                            w1t = []
                            for kc in range(NKC):
                                wt = wsp.tile([P, 1024], F32R, tag="w",
                                              name="w")
                                nc.sync.dma_start(
                                    wt[:], dr["w1"][kc][:, cg * 1024:
                                                        (cg + 1) * 1024])
                                w1t.append(wt)
                            for cc in range(NKC):
                                cidx = cg * 8 + cc
                                ps = ps_f1.tile([P, 512], F32, tag="proj0",
                                                name="proj0")
                                for kc in range(NKC):
                                    nc.tensor.matmul(
                                        ps[:], w1t[kc][:, cc * P:(cc + 1) * P],
                                        h3[kc], start=(kc == 0),
                                        stop=(kc == NKC - 1))
                                nc.scalar.activation(
                                    aT[cidx], ps[:], AF.Relu,
                                    bias=bias["b1"][:, cidx:cidx + 1])

                with ExitStack() as yscope:
                    ps_y = yscope.enter_context(
                        tc.tile_pool(name=f"ps_y{_rep}", bufs=1, space="PSUM"))
                    yps = [ps_y.tile([P, 512], F32, tag=f"y{c2}", name=f"y{c2}")
                           for c2 in range(NKC)]
                    for kc2 in range(NFC):
                        wt = w2p.tile([P, D], F32R, tag="w", name="w")
                        nc.sync.dma_start(wt[:], dr["w2"][kc2])
                        for c2 in range(NKC):
                            nc.tensor.matmul(
                                yps[c2][:], wt[:, c2 * P:(c2 + 1) * P],
                                aT[kc2], start=(kc2 == 0),
                                stop=(kc2 == NFC - 1))
                    for c2 in range(NKC):
                        nc.vector.scalar_tensor_tensor(
                            xcur[c2][:], yps[c2][:], bias["b2"][:, c2:c2 + 1],
                            xcur[c2][:], op0=ALU.add, op1=ALU.add)

            for c2 in range(NKC):
                nc.sync.dma_start(outT[c2 * P:(c2 + 1) * P, :], xcur[c2][:])

    nc.finalize()
    return nc


# ---------------------------------------------------------------------------
# host side
# ---------------------------------------------------------------------------

def _tile4(wT):
    """[D_in, D_out] -> [D_in/P, D_out/P, P, P] contiguous tiles."""
    di, do = wT.shape
    return np.ascontiguousarray(
        wT.reshape(di // P, P, do // P, P).transpose(0, 2, 1, 3))


def host_prep(inputs):
    f32 = np.float32
    x = np.asarray(inputs["x"], f32)
    mem = np.asarray(inputs["memory"], f32)
    tgt = np.asarray(inputs["tgt_mask"])
    src = np.asarray(inputs["src_mask"])

    add_tgt = (tgt.astype(f32) - 1.0) * 1e9     # [B, T, T]: 0 or -1e9
    add_src = (src.astype(f32) - 1.0) * 1e9     # [B, T]

    shared = {"ones_r": np.ones((P, P), f32)}
    sm_cols = []
    for pre in ("sa", "ca"):
        wq = np.asarray(inputs[f"{pre}_wq"], f32)
        shared[f"{pre}_wq"] = np.ascontiguousarray(
            (wq.T * 0.125).reshape(NKC, P, D))
        shared[f"{pre}_wk"] = np.ascontiguousarray(
            np.asarray(inputs[f"{pre}_wk"], f32).T.reshape(NKC, P, D))
        shared[f"{pre}_wo"] = np.ascontiguousarray(
            np.asarray(inputs[f"{pre}_wo"], f32).T.reshape(NKC, P, D))
        shared[f"{pre}_wv"] = np.ascontiguousarray(
            np.asarray(inputs[f"{pre}_wv"], f32).T.reshape(NKC, P, D))
        shared[f"{pre}_bv"] = np.asarray(inputs[f"{pre}_bv"], f32)
    for pre in ("sa", "ca"):
        sm_cols.append((np.asarray(inputs[f"{pre}_bq"], f32)
                        * 0.125).reshape(NKC, P).T)
        sm_cols.append(np.asarray(inputs[f"{pre}_bk"],
                                  f32).reshape(NKC, P).T)
        sm_cols.append(np.asarray(inputs[f"{pre}_bo"],
                                  f32).reshape(NKC, P).T)
    shared["lnwrows"] = np.concatenate(
        [np.asarray(inputs[f"{ln}_w"], f32) for ln in
         ("ln1", "ln2", "ln3")]).reshape(1, 3 * D)
    shared["w1"] = np.ascontiguousarray(
        np.asarray(inputs["ff_w1"], f32).T.reshape(NKC, P, DFF))
    shared["w2"] = np.ascontiguousarray(
        np.asarray(inputs["ff_w2"], f32).T.reshape(NFC, P, D))
    # order must match sa_bq/sa_bk/sa_bo, ca_bq/ca_bk/ca_bo above
    sm_fixed = [sm_cols[0], sm_cols[1], sm_cols[2],
                sm_cols[3], sm_cols[4], sm_cols[5],
                np.asarray(inputs["ff_b1"], f32).reshape(NFC, P).T,
                np.asarray(inputs["ff_b2"], f32).reshape(NKC, P).T]
    for ln in ("ln1", "ln2", "ln3"):
        sm_fixed.append(np.asarray(inputs[f"{ln}_b"],
                                   f32).reshape(NKC, P).T)


    maps = []
    for c in range(8):
        b, half = c // 2, c % 2
        q0 = half * OWN
        order = np.concatenate(
            [np.arange(q0, q0 + OWN), np.r_[0:q0, q0 + OWN:T]]).astype(
                np.int64)
        m = dict(shared)
        m["xT"] = np.ascontiguousarray(x[b][order].T)
        m["memT"] = np.ascontiguousarray(mem[b].T)
        mt = np.ascontiguousarray(add_tgt[b][q0:q0 + OWN][:, order].T)
        m["maskT"] = mt
        smask_c = add_src[b].reshape(NSC, P).T
        tailb_c = mt[OWN:, 0].reshape(4, P).T
        m["smalls"] = np.ascontiguousarray(
            np.concatenate(sm_fixed + [smask_c, tailb_c], axis=1))
        maps.append(m)
    return maps


def _tail_rows_constant(maps):
    """True when every core's mask s-chunks 4..7 are constant per s-row, so
    they can be applied as a per-partition exp bias instead of tensor adds."""
    for m in maps:
        tail = m["maskT"][OWN:, :]
        if not np.all(tail == tail[:, :1]):
            return False
    return True


def gather(results):
    out = np.zeros((B, T, D), np.float32)
    for c in range(8):
        b, half = c // 2, c % 2
        out[b, half * OWN:(half + 1) * OWN, :] = results[c]["outT"].T
    return out


_NC_CACHE = {}


def kernel(**inputs):
    in_maps = host_prep(inputs)
    mode = "split" if _tail_rows_constant(in_maps) else "full"
    if mode not in _NC_CACHE:
        _NC_CACHE[mode] = build_program(mask_mode=mode)
    nc = _NC_CACHE[mode]
    res = run_bass_kernel_spmd(nc, in_maps, list(range(8)))
    return gather(res.results)


if __name__ == "__main__":
    import reference as ref_mod
    inputs = {k: np.asarray(v) for k, v in ref_mod.setup_inputs().items()}
    expected = np.asarray(ref_mod.reference(**ref_mod.setup_inputs()))
    actual = kernel(**inputs)
    err = np.abs(actual - expected).max()
    rel = err / np.abs(expected).max()
    print("max abs err:", err, "rel:", rel)



# revision 17
# speedup vs baseline: 2.3017x; 2.3017x over previous
"""Transformer decoder layer (pre-norm, self-attn + cross-attn + FFN) on 8
Trainium2 NeuronCores.

Sharding: core c handles batch b = c//2 and the contiguous half of the 1024
target tokens h = c%2 (512 query rows each). K/V work is duplicated within
each batch pair so there are no collectives; every core runs an identical
program on different data. The host rotates the token order per core so that
each core's own tokens are always columns [0, 512) -> one uniform SPMD
program.

On-device layout is feature-major ([d_model, token]) throughout, so no
on-device transposes are needed: the host pre-transposes x / memory / weights
and transposes the output back. Masks are converted to additive fp32 masks on
the host. Matmuls run in float32r (full PE rate at free dim 512).
"""

import numpy as np
from contextlib import ExitStack

import concourse.bass as bass
import concourse.bacc as bacc
import concourse.tile as tile
from concourse import mybir
from concourse.bass_utils import run_bass_kernel_spmd

D = 1024        # d_model
H = 16          # heads
DK = 64         # head dim
DFF = 4096
B = 4
T = 1024        # tgt/src len
OWN = 512       # query rows per core
P = 128         # partitions
NKC = D // P    # 8 feature chunks
NSC = T // P    # 8 s-chunks
NFC = DFF // P  # 32 ffn chunks
EPS = 1e-6

F32 = mybir.dt.float32
F32R = mybir.dt.float32r
F8 = mybir.dt.float8e4
DR = mybir.MatmulPerfMode.DoubleRow
AF = mybir.ActivationFunctionType
ALU = mybir.AluOpType


def _r(ap):
    return ap.bitcast(F32R)


# ---------------------------------------------------------------------------
# program builder (identical for every core; only DRAM contents differ)
# ---------------------------------------------------------------------------

def build_program(repeat=1, mask_mode="split"):
    nc = bacc.Bacc(None)
    dr = {}

    def din(name, shape, dt=F32):
        dr[name] = nc.dram_tensor(name, list(shape), dt, kind="ExternalInput")
        return dr[name]

    din("ones_r", [P, P], F32R)
    din("xT", [D, T], F32R)                # batch-b x, transposed, own tokens first
    din("memT", [D, T], F32R)              # memory[b] transposed
    din("maskT", [T, OWN])           # additive tgt mask, [s_rot, q_local]
    din("lnwrows", [1, 3 * D], F32R)  # LN w rows, concat (PE bcast)
    # all small per-partition tensors packed into one DMA:
    # cols: [sa_bq 8][sa_bk 8][sa_bo 8][ca_bq 8][ca_bk 8][ca_bo 8]
    #       [b1 32][b2 8][ln1_b 8][ln2_b 8][ln3_b 8][smask 8][tailb 4]
    din("smalls", [P, 124])
    for pre in ("sa", "ca"):
        din(f"{pre}_wq", [NKC // 2, P, 2, D], F8)   # x32 scaled fp8, k-pairs
        din(f"{pre}_wk", [NKC // 2, P, 2, D], F8)
        din(f"{pre}_wv", [NKC // 2, P, 2, D], F8)
        din(f"{pre}_wo", [NKC, P, D], F32R)
        din(f"{pre}_bv", [D], F32R)      # placed at partition 64 (outer trick)
    din("memT8", [NKC // 2, P, 2, T], F8)  # fp8 memory, k-pairs
    din("w1", [NKC, P, DFF], F32R)
    din("w2", [NFC, P, D], F32R)

    outT = nc.dram_tensor("outT", [D, OWN], F32, kind="ExternalOutput")

    with ExitStack() as ctx:
        tc = ctx.enter_context(tile.TileContext(nc))
        ctx.enter_context(nc.allow_low_precision(
            reason="float32r staging for full-rate PE matmuls"))
        persist = ctx.enter_context(tc.tile_pool(name="persist", bufs=1))

        ones = persist.tile([P, P], F32R, tag="ones", name="ones")
        nc.gpsimd.dma_start(ones[:], dr["ones_r"][:])
        ones_f = persist.tile([P, 1], F32, tag="ones_f", name="ones_f")
        nc.vector.memset(ones_f[:], 1.0)
        eps_11 = persist.tile([1, 1], F32, tag="eps11", name="eps11")
        nc.vector.memset(eps_11[:], EPS)

        smalls = persist.tile([P, 124], F32, tag="smalls", name="smalls")
        nc.gpsimd.dma_start(smalls[:], dr["smalls"][:])
        _off = [0]

        def s_col(n):
            t = smalls[:, _off[0]:_off[0] + n]
            _off[0] += n
            return t

        bias = {}
        for pre in ("sa", "ca"):
            for nm in ("bq", "bk", "bo"):
                bias[f"{pre}_{nm}"] = s_col(NKC)
        bias["b1"] = s_col(NFC)
        bias["b2"] = s_col(NKC)
        lnp = {}
        for ln in ("ln1", "ln2", "ln3"):
            lnp[f"{ln}_b"] = s_col(NKC)
        smask = s_col(NSC)
        tailb = s_col(4)
        for pre in ("sa", "ca"):
            bv = persist.tile([P, D], F32R, tag=f"{pre}_bv", name=f"{pre}_bv")
            nc.gpsimd.dma_start(bv[64:65, :], dr[f"{pre}_bv"][None, :])
            bias[f"{pre}_bv"] = bv
        lnwr = persist.tile([1, 3 * D], F32R, tag="lnwrows", name="lnwrows")
        nc.gpsimd.dma_start(lnwr[:], dr["lnwrows"][:])
        lnrow = {"ln1": lnwr[0:1, 0:D], "ln2": lnwr[0:1, D:2 * D],
                 "ln3": lnwr[0:1, 2 * D:3 * D]}

        # residual (own tokens), lives to the end
        xcur = [persist.tile([P, OWN], F32, tag=f"xc{i}", name=f"xc{i}")
                for i in range(NKC)]

        tmp = ctx.enter_context(tc.tile_pool(name="tmp", bufs=2))
        stats = ctx.enter_context(tc.tile_pool(name="stats", bufs=1))

        # ------------------------------------------------------------------
        def ln_sums(blk, src_get, ps_st, nbuf=1):
            """Pass 1: PE ones-reductions for sum and sum-of-squares."""
            sb0 = src_get(blk, 0)
            src_is_r = sb0.dtype == F32R
            ones_s = ones[:, 0:1] if src_is_r else ones_f[:]
            ps_s = ps_st.tile([1, 512], F32, tag="lns", name="lns", bufs=nbuf)
            ps_q = ps_st.tile([1, 512], F32, tag="lnq", name="lnq", bufs=nbuf)
            for kc in range(NKC):
                sbk = sb0 if kc == 0 else src_get(blk, kc)
                nc.tensor.matmul(ps_s[:], ones_s, sbk,
                                 start=(kc == 0), stop=(kc == NKC - 1))
                sq = tmp.tile([P, 512], F32R, tag="lnsq", name="lnsq", bufs=1)
                nc.scalar.activation(sq[:], sbk, AF.Square)
                nc.tensor.matmul(ps_q[:], ones[:, 0:1], sq[:],
                                 start=(kc == 0), stop=(kc == NKC - 1))
            return ps_s, ps_q

        def ln_normalize(blk, src_get, db, wrow, b_pn, ps_st, ps_sq):
            """Pass 2: stats chain (DVE/Act) + PE broadcasts + apply."""
            ps_s, ps_q = ps_sq
            s2 = stats.tile([1, 512], F32, tag="lnstA", name="lnstA")
            # s2 = (sum/sqrt(D*(D-1)))^2 = sum^2/(D*(D-1))
            nc.scalar.activation(s2[:], ps_s[:], AF.Square,
                                 scale=float(1.0 / np.sqrt(D * (D - 1.0))))
            var = stats.tile([1, 512], F32, tag="lnstB", name="lnstB")
            nc.vector.scalar_tensor_tensor(
                var[:], ps_q[:], 1.0 / (D - 1.0), s2[:],
                op0=ALU.mult, op1=ALU.subtract)
            std = stats.tile([1, 512], F32, tag="lnstA", name="lnstA")
            nc.scalar.activation(std[:], var[:], AF.Sqrt)
            nc.scalar.add(std[:], std[:], eps_11[:])
            rstd = stats.tile([1, 512], F32R, tag="lnstC", name="lnstC")
            nc.vector.reciprocal(rstd[:], std[:])
            mr = stats.tile([1, 512], F32R, tag="lnstB", name="lnstB")
            nc.vector.scalar_tensor_tensor(
                mr[:], ps_s[:], 1.0 / D, rstd[:],
                op0=ALU.mult, op1=ALU.mult)
            for kc in range(NKC):
                wl = wrow[0:1, kc * P:(kc + 1) * P]
                ps_rb = ps_st.tile([P, 512], F32, tag="ln_rb",
                                   name="ln_rb", bufs=1)
                nc.tensor.matmul(ps_rb[:], wl, rstd[:],
                                 start=True, stop=True)
                ps_mb = ps_st.tile([P, 512], F32, tag="ln_mb",
                                   name="ln_mb", bufs=1)
                nc.tensor.matmul(ps_mb[:], wl, mr[:],
                                 start=True, stop=True)
                t = tmp.tile([P, 512], F32, tag="lnt", name="lnt")
                nc.vector.tensor_mul(t[:], src_get(blk, kc), ps_rb[:])
                nc.vector.scalar_tensor_tensor(
                    db[kc], t[:], b_pn[:, kc:kc + 1], ps_mb[:],
                    op0=ALU.add, op1=ALU.subtract)

        def layer_norm(nblk, src_get, dst_blocks, wrow, b_pn, ps_st,
                       mid=None):
            """Feature-major LN, h = (x - mean) * (w * rstd) + b.
            All blocks' sum passes issue first (so block b+1's PE sums hide
            block b's DVE stats chain), then `mid()` may issue extra PE work,
            then the normalize passes run."""
            if mid is None:
                for blk in range(nblk):
                    s = ln_sums(blk, src_get, ps_st)
                    ln_normalize(blk, src_get, dst_blocks[blk], wrow, b_pn,
                                 ps_st, s)
            else:
                sums = [ln_sums(blk, src_get, ps_st, nbuf=nblk)
                        for blk in range(nblk)]
                mid()
                for blk in range(nblk):
                    ln_normalize(blk, src_get, dst_blocks[blk], wrow, b_pn,
                                 ps_st, sums[blk])

        # ------------------------------------------------------------------
        def load_w_rows(wpool, wname, n=NKC, cols=D):
            tiles = []
            for kc in range(n):
                wt = wpool.tile([P, cols], F32R, tag="w", name="w")
                nc.sync.dma_start(wt[:], dr[wname][kc])
                tiles.append(wt)
            return tiles

        def load_w_pairs(wpool, wname, n=NKC // 2, cols=D):
            """fp8 weight k-pair tiles [P, 2, cols] for DoubleRow."""
            tiles = []
            for kp in range(n):
                wt = wpool.tile([P, 2, cols], F8, tag="w8", name="w8")
                nc.sync.dma_start(wt[:], dr[wname][kp])
                tiles.append(wt)
            return tiles

        def proj_fm8(wpool, wname, bias_pn, src_blocks, dst, ps_acc, scl):
            """fp8 DoubleRow feature-major projection.
            dst[c][:, blk*512:..] = scl * sum_kp W8[kp,c].T @ src8[blk][kp] + b
            src8 blocks are lists of [P, 2, 512] fp8 pair tiles."""
            nblk = len(src_blocks)
            w_tiles = load_w_pairs(wpool, wname)
            for c in range(NKC):
                pss = [ps_acc.tile([P, 512], F32, tag=f"proj{blk}",
                                   name=f"proj{blk}") for blk in range(nblk)]
                for kp in range(NKC // 2):
                    for blk in range(nblk):
                        nc.tensor.matmul(pss[blk][:],
                                         w_tiles[kp][:, :, c * P:(c + 1) * P],
                                         src_blocks[blk][kp][:],
                                         start=(kp == 0),
                                         stop=(kp == NKC // 2 - 1),
                                         perf_mode=DR)
                for blk in range(nblk):
                    nc.vector.tensor_scalar(
                        dst[c][:, blk * 512:(blk + 1) * 512],
                        pss[blk][:], scl, bias_pn[:, c:c + 1],
                        op0=ALU.mult, op1=ALU.add)

        def proj_tm_vaug8(wpool, wname, src_blocks, vaug, ps_acc, scl):
            """fp8 DoubleRow token-major V projection into vaug tiles."""
            wv = load_w_pairs(wpool, wname)
            for dc in range(2):
                for st in range(NSC):
                    sb = src_blocks[st // 4]
                    t0 = (st % 4) * P
                    ps = ps_acc.tile([P, 512], F32, tag="proj0", name="proj0")
                    for kp in range(NKC // 2):
                        nc.tensor.matmul(
                            ps[:], sb[kp][:, :, t0:t0 + P],
                            wv[kp][:, :, dc * 512:(dc + 1) * 512],
                            start=(kp == 0), stop=(kp == NKC // 2 - 1),
                            perf_mode=DR)
                    nc.vector.tensor_scalar_mul(
                        vaug[st][:, 8 * dc:8 * (dc + 1), 0:DK],
                        ps[:].rearrange("p (h d) -> p h d", h=8), scl)

        # ------------------------------------------------------------------
        def attention(KT, QT, vaug, OT, mask_tiles, tail_pn, smask_pn,
                      bv_tile, att_pools):
            """Software-pipelined: scores for step s+1 issue on the PE before
            the AV matmuls of step s, so the DVE/Act exp latency of step s
            hides under PE work.  PSUM: 4 score banks + 2 po + 1 rb = 7."""
            ps_sc, ps_av, ps_rb, epool = att_pools
            nmask = len(mask_tiles) if mask_tiles is not None else 0

            def issue_scores(c, i):
                pair = [ps_sc.tile([P, 512], F32, tag="sc", name="sc", bufs=4),
                        ps_sc.tile([P, 512], F32, tag="sc", name="sc", bufs=4)]
                for h01 in (0, 1):
                    sl = slice(64 * h01, 64 * h01 + 64)
                    nc.tensor.matmul(
                        pair[h01][:], KT[c][sl, i * P:(i + 1) * P],
                        QT[c][sl, :], start=True, stop=True)
                return pair

            def issue_exp(i, pssc_h):
                e = epool.tile([P, 512], F32R, tag="e", name="e")
                if i < nmask:
                    tm = epool.tile([P, 512], F32, tag="emask",
                                    name="emask", bufs=2)
                    nc.vector.tensor_add(tm[:], pssc_h[:], mask_tiles[i][:])
                    nc.scalar.activation(e[:], tm[:], AF.Exp)
                elif tail_pn is not None:
                    nc.scalar.activation(e[:], pssc_h[:], AF.Exp,
                                         bias=tail_pn[:, i - 4:i - 3])
                elif smask_pn is not None:
                    nc.scalar.activation(e[:], pssc_h[:], AF.Exp,
                                         bias=smask_pn[:, i:i + 1])
                else:
                    nc.scalar.activation(e[:], pssc_h[:], AF.Exp)
                return e

            seq = [(c, i) for c in range(NKC) for i in range(NSC)]
            po_c = None
            pair = issue_scores(*seq[0])
            for idx, (c, i) in enumerate(seq):
                if i == 0:
                    po_c = [ps_av.tile([DK + 1, 512], F32, tag="po",
                                       name="po", bufs=3),
                            ps_av.tile([DK + 1, 512], F32, tag="po",
                                       name="po", bufs=3)]
                es = [issue_exp(i, pair[0]), issue_exp(i, pair[1])]
                if idx + 1 < len(seq):
                    pair = issue_scores(*seq[idx + 1])
                for h01 in (0, 1):
                    nc.tensor.matmul(
                        po_c[h01][:], vaug[i][:, 2 * c + h01, :],
                        es[h01][:], start=(i == 0), stop=(i == NSC - 1))
                if i == NSC - 1:
                    for h01 in (0, 1):
                        h = 2 * c + h01
                        po = po_c
                        sums = epool.tile([P, 512], F32R, tag="sums",
                                          name="sums", bufs=2)
                        nc.vector.tensor_copy(sums[64:65, :],
                                              po[h01][DK:DK + 1, :])
                        # O_un += bv (x) sums (folds V bias through softmax)
                        nc.tensor.matmul(
                            po[h01][0:DK, :],
                            bv_tile[64:65, DK * h:DK * h + DK],
                            sums[64:65, :], start=False, stop=True,
                            skip_group_check=True)
                        nc.vector.reciprocal(sums[64:65, :], sums[64:65, :])
                        prb = ps_rb.tile([DK, 512], F32, tag="rb", name="rb")
                        nc.tensor.matmul(prb[:], ones[64:65, 0:DK],
                                         sums[64:65, :], start=True, stop=True)
                        # DVE can read only one PSUM operand: stage po first
                        o_un = epool.tile([DK, 512], F32, tag="o_un",
                                          name="o_un", bufs=2)
                        nc.vector.tensor_copy(o_un[:], po[h01][0:DK, :])
                        if h01 == 0:
                            nc.vector.tensor_mul(OT[c][0:DK, :], o_un[:],
                                                 prb[:])
                        else:
                            ot = epool.tile([DK, 512], F32R, tag="ot",
                                            name="ot", bufs=1)
                            nc.vector.tensor_mul(ot[:], o_un[:], prb[:])
                            nc.sync.dma_start(OT[c][DK:P, :], ot[:])

        def out_proj_residual(w_tiles, bias_pn, OT, ps_acc):
            for c2 in range(NKC):
                ps = ps_acc.tile([P, 512], F32, tag="proj0", name="proj0")
                for c in range(NKC):
                    nc.tensor.matmul(ps[:],
                                     w_tiles[c][:, c2 * P:(c2 + 1) * P],
                                     OT[c][:],
                                     start=(c == 0), stop=(c == NKC - 1))
                nc.vector.scalar_tensor_tensor(
                    xcur[c2][:], ps[:], bias_pn[:, c2:c2 + 1], xcur[c2][:],
                    op0=ALU.add, op1=ALU.add)

        def attention_block(prefix, pn, KT, QT, vaug, mask_tiles, tail_pn,
                            use_smask, scope):
            """Runs attention + output projection + residual."""
            otp = scope.enter_context(
                tc.tile_pool(name=f"{pn}_otp", bufs=1))
            OT = [otp.tile([P, OWN], F32R, tag=f"OT{c}", name=f"OT{c}")[:]
                  for c in range(NKC)]
            # wo prefetch overlaps the attention phase
            wsp = scope.enter_context(tc.tile_pool(name=f"{pn}_wso", bufs=8))
            wo_tiles = load_w_rows(wsp, f"{prefix}_wo")
            with ExitStack() as att:
                ps_sc = att.enter_context(
                    tc.tile_pool(name=f"{pn}_psc", bufs=4, space="PSUM"))
                ps_av = att.enter_context(
                    tc.tile_pool(name=f"{pn}_pav", bufs=2, space="PSUM"))
                ps_rb = att.enter_context(
                    tc.tile_pool(name=f"{pn}_prb", bufs=1, space="PSUM"))
                epool = att.enter_context(
                    tc.tile_pool(name=f"{pn}_ep", bufs=3))
                attention(KT, QT, vaug, OT, mask_tiles, tail_pn,
                          smask if use_smask else None,
                          bias[f"{prefix}_bv"], (ps_sc, ps_av, ps_rb, epool))
            with tc.tile_pool(name=f"{pn}_pso", bufs=4, space="PSUM") as pso:
                out_proj_residual(wo_tiles, bias[f"{prefix}_bo"], OT, pso)

        for _rep in range(repeat):
            # ==================================================================
            # phase 1+2: LN1, self-attention
            # ==================================================================
            with ExitStack() as sa:
                big = sa.enter_context(tc.tile_pool(name=f"sa_big{_rep}", bufs=1))
                QT = [big.tile([P, OWN], F32R, tag=f"QT{c}", name=f"QT{c}")[:]
                      for c in range(NKC)]
                KT = [big.tile([P, T], F32R, tag=f"KT{c}", name=f"KT{c}")[:]
                      for c in range(NKC)]
                vaug = [big.tile([P, H, DK + 1], F32R, tag=f"V{i}", name=f"V{i}")[:]
                        for i in range(NSC)]
                for i in range(NSC):
                    nc.vector.memset(vaug[i][:, :, DK].bitcast(F32), 1.0)

                with ExitStack() as ph:
                    h1p = ph.enter_context(
                        tc.tile_pool(name=f"h1{_rep}", bufs=1))
                    h1_blocks = [
                        [h1p.tile([P, 2, 512], F8, tag=f"h1_{blk}_{kp}",
                                  name=f"h1_{blk}_{kp}")[:]
                         for kp in range(NKC // 2)]
                        for blk in range(2)]
                    h1_dst = [
                        [h1_blocks[blk][kc // 2][:, kc % 2, :]
                         for kc in range(NKC)]
                        for blk in range(2)]
                    with ExitStack() as wscope:
                        # weight pool open during LN1 so the Q/K/V weight
                        # prefetch overlaps the LN chain
                        wsp = wscope.enter_context(
                            tc.tile_pool(name=f"sa_ws{_rep}", bufs=9))
                        psa = wscope.enter_context(
                            tc.tile_pool(name=f"ps_sap{_rep}", bufs=2,
                                         space="PSUM"))
                        with ExitStack() as lnscope:
                            xsp = lnscope.enter_context(
                                tc.tile_pool(name=f"xs{_rep}", bufs=3))
                            ps_ln = lnscope.enter_context(
                                tc.tile_pool(name=f"ps_ln1{_rep}", bufs=1,
                                             space="PSUM"))

                            def x_get(blk, kc):
                                xt = xsp.tile([P, 512], F32R, tag="xs",
                                              name="xs")
                                nc.sync.dma_start(
                                    xt[:],
                                    dr["xT"][kc * P:(kc + 1) * P,
                                             blk * 512:(blk + 1) * 512])
                                return xt[:]

                            layer_norm(2, x_get, h1_dst, lnrow["ln1"],
                                       lnp["ln1_b"], ps_ln)
                        proj_fm8(wsp, "sa_wq", bias["sa_bq"], [h1_blocks[0]],
                                 QT, psa, 1.0 / 256.0)
                        proj_fm8(wsp, "sa_wk", bias["sa_bk"], h1_blocks,
                                 KT, psa, 1.0 / 32.0)
                        proj_tm_vaug8(wsp, "sa_wv", h1_blocks, vaug, psa,
                                      1.0 / 32.0)

                # residual base (own half of x)
                for kc in range(NKC):
                    nc.sync.dma_start(
                        xcur[kc][:],
                        dr["xT"][kc * P:(kc + 1) * P, 0:OWN].bitcast(F32))

                with ExitStack() as mscope:
                    mp = mscope.enter_context(
                        tc.tile_pool(name=f"maskp{_rep}", bufs=1))
                    nmt = 4 if mask_mode == "split" else NSC
                    mask_tiles = []
                    for i in range(nmt):
                        mt = mp.tile([P, OWN], F32, tag=f"mask{i}",
                                     name=f"mask{i}")
                        nc.sync.dma_start(mt[:],
                                          dr["maskT"][i * P:(i + 1) * P, :])
                        mask_tiles.append(mt[:])
                    attention_block(
                        "sa", f"sa{_rep}", KT, QT, vaug, mask_tiles,
                        tailb if mask_mode == "split" else None,
                        False, mscope)

            # ==================================================================
            # phase 3: cross attention
            # ==================================================================
            with ExitStack() as ca:
                big = ca.enter_context(tc.tile_pool(name=f"ca_big{_rep}", bufs=1))
                QT = [big.tile([P, OWN], F32R, tag=f"cQT{c}", name=f"cQT{c}")[:]
                      for c in range(NKC)]
                KT = [big.tile([P, T], F32R, tag=f"cKT{c}", name=f"cKT{c}")[:]
                      for c in range(NKC)]
                vaug = [big.tile([P, H, DK + 1], F32R, tag=f"cV{i}",
                                 name=f"cV{i}")[:] for i in range(NSC)]
                for i in range(NSC):
                    nc.vector.memset(vaug[i][:, :, DK].bitcast(F32), 1.0)

                # K/V projections first: they depend only on memory, so
                # they keep the PE busy while the LN2 chain runs.
                with ExitStack() as mm_scope:
                    wsp = mm_scope.enter_context(
                        tc.tile_pool(name=f"ca_wkv{_rep}", bufs=9))
                    pca = mm_scope.enter_context(
                        tc.tile_pool(name=f"ps_ckv{_rep}", bufs=2,
                                     space="PSUM"))
                    with ExitStack() as mscope2:
                        memp = mscope2.enter_context(
                            tc.tile_pool(name=f"mem{_rep}", bufs=1))
                        mem_blocks = []
                        for blk in range(2):
                            mb = []
                            for kp in range(NKC // 2):
                                mt = memp.tile([P, 2, 512], F8,
                                               tag=f"m{blk}_{kp}",
                                               name=f"m{blk}_{kp}")
                                nc.sync.dma_start(
                                    mt[:],
                                    dr["memT8"][kp][:, :,
                                                    blk * 512:(blk + 1) * 512])
                                mb.append(mt[:])
                            mem_blocks.append(mb)

                        with ExitStack() as ph:
                            h2p = ph.enter_context(
                                tc.tile_pool(name=f"h2{_rep}", bufs=1))
                            h2 = [h2p.tile([P, 2, OWN], F8, tag=f"h2_{kp}",
                                           name=f"h2_{kp}")[:]
                                  for kp in range(NKC // 2)]
                            h2_dst = [h2[kc // 2][:, kc % 2, :]
                                      for kc in range(NKC)]

                            def ca_kv_mid():
                                proj_fm8(wsp, "ca_wk", bias["ca_bk"],
                                         mem_blocks, KT, pca, 1.0 / 32.0)
                                proj_tm_vaug8(wsp, "ca_wv", mem_blocks,
                                              vaug, pca, 1.0 / 32.0)

                            with tc.tile_pool(name=f"ps_ln2{_rep}", bufs=1,
                                              space="PSUM") as psl:
                                layer_norm(1, lambda blk, kc: xcur[kc][:],
                                           [h2_dst], lnrow["ln2"],
                                           lnp["ln2_b"], psl, mid=ca_kv_mid)
                            proj_fm8(wsp, "ca_wq", bias["ca_bq"], [h2], QT,
                                     pca, 1.0 / 256.0)

                with ExitStack() as ascope:
                    attention_block("ca", f"ca{_rep}", KT, QT, vaug, None,
                                None, True, ascope)

            # ==================================================================
            # phase 4: FFN
            # ==================================================================
            with ExitStack() as ff:
                ap_pool = ff.enter_context(tc.tile_pool(name=f"aT{_rep}", bufs=1))
                w2p = ff.enter_context(tc.tile_pool(name=f"w2p{_rep}", bufs=6))
                aT = [ap_pool.tile([P, OWN], F32R, tag=f"aT{i}", name=f"aT{i}")[:]
                      for i in range(NFC)]
                with ExitStack() as ph:
                    h3p = ph.enter_context(
                        tc.tile_pool(name=f"h3{_rep}", bufs=1))
                    h3 = [h3p.tile([P, OWN], F32R, tag=f"h3_{kc}",
                                   name=f"h3_{kc}")[:] for kc in range(NKC)]
                    with ExitStack() as wscope:
                        wsp = wscope.enter_context(
                            tc.tile_pool(name=f"ff_ws{_rep}", bufs=10))
                        ps_f1 = wscope.enter_context(
                            tc.tile_pool(name=f"ps_ff1{_rep}", bufs=4,
                                         space="PSUM"))
                        with tc.tile_pool(name=f"ps_ln3{_rep}", bufs=1,
                                          space="PSUM") as psl:
                            layer_norm(1, lambda blk, kc: xcur[kc][:], [h3],
                                       lnrow["ln3"], lnp["ln3_b"], psl)
                        for cg in range(4):
                            w1t = []
                            for kc in range(NKC):
                                wt = wsp.tile([P, 1024], F32R, tag="w",
                                              name="w")
                                nc.sync.dma_start(
                                    wt[:], dr["w1"][kc][:, cg * 1024:
                                                        (cg + 1) * 1024])
                                w1t.append(wt)
                            for cc in range(NKC):
                                cidx = cg * 8 + cc
                                ps = ps_f1.tile([P, 512], F32, tag="proj0",
                                                name="proj0")
                                for kc in range(NKC):
                                    nc.tensor.matmul(
                                        ps[:], w1t[kc][:, cc * P:(cc + 1) * P],
                                        h3[kc], start=(kc == 0),
                                        stop=(kc == NKC - 1))
                                nc.scalar.activation(
                                    aT[cidx], ps[:], AF.Relu,
                                    bias=bias["b1"][:, cidx:cidx + 1])

                with ExitStack() as yscope:
                    ps_y = yscope.enter_context(
                        tc.tile_pool(name=f"ps_y{_rep}", bufs=1, space="PSUM"))
                    yps = [ps_y.tile([P, 512], F32, tag=f"y{c2}", name=f"y{c2}")
                           for c2 in range(NKC)]
                    for kc2 in range(NFC):
                        wt = w2p.tile([P, D], F32R, tag="w", name="w")
                        nc.sync.dma_start(wt[:], dr["w2"][kc2])
                        for c2 in range(NKC):
                            nc.tensor.matmul(
                                yps[c2][:], wt[:, c2 * P:(c2 + 1) * P],
                                aT[kc2], start=(kc2 == 0),
                                stop=(kc2 == NFC - 1))
                    for c2 in range(NKC):
                        nc.vector.scalar_tensor_tensor(
                            xcur[c2][:], yps[c2][:], bias["b2"][:, c2:c2 + 1],
                            xcur[c2][:], op0=ALU.add, op1=ALU.add)

            for c2 in range(NKC):
                nc.sync.dma_start(outT[c2 * P:(c2 + 1) * P, :], xcur[c2][:])

    nc.finalize()
    return nc


# ---------------------------------------------------------------------------
# host side
# ---------------------------------------------------------------------------

def _tile4(wT):
    """[D_in, D_out] -> [D_in/P, D_out/P, P, P] contiguous tiles."""
    di, do = wT.shape
    return np.ascontiguousarray(
        wT.reshape(di // P, P, do // P, P).transpose(0, 2, 1, 3))


def host_prep(inputs):
    f32 = np.float32
    x = np.asarray(inputs["x"], f32)
    mem = np.asarray(inputs["memory"], f32)
    tgt = np.asarray(inputs["tgt_mask"])
    src = np.asarray(inputs["src_mask"])

    add_tgt = (tgt.astype(f32) - 1.0) * 1e9     # [B, T, T]: 0 or -1e9
    add_src = (src.astype(f32) - 1.0) * 1e9     # [B, T]

    import ml_dtypes
    f8 = ml_dtypes.float8_e4m3

    def pack8(wT):
        # [D, cols] -> fp8 x32 k-pair tiles [NKC//2, P, 2, cols]
        d, cols = wT.shape
        return np.ascontiguousarray(
            (wT * 32.0).reshape(d // P // 2, 2, P, cols)
            .transpose(0, 2, 1, 3)).astype(f8)

    shared = {"ones_r": np.ones((P, P), f32)}
    sm_cols = []
    for pre in ("sa", "ca"):
        wq = np.asarray(inputs[f"{pre}_wq"], f32)
        shared[f"{pre}_wq"] = pack8(wq.T)
        shared[f"{pre}_wk"] = pack8(np.asarray(inputs[f"{pre}_wk"], f32).T)
        shared[f"{pre}_wv"] = pack8(np.asarray(inputs[f"{pre}_wv"], f32).T)
        shared[f"{pre}_wo"] = np.ascontiguousarray(
            np.asarray(inputs[f"{pre}_wo"], f32).T.reshape(NKC, P, D))
        shared[f"{pre}_bv"] = np.asarray(inputs[f"{pre}_bv"], f32)
    for pre in ("sa", "ca"):
        sm_cols.append((np.asarray(inputs[f"{pre}_bq"], f32)
                        * 0.125).reshape(NKC, P).T)
        sm_cols.append(np.asarray(inputs[f"{pre}_bk"],
                                  f32).reshape(NKC, P).T)
        sm_cols.append(np.asarray(inputs[f"{pre}_bo"],
                                  f32).reshape(NKC, P).T)
    shared["lnwrows"] = np.concatenate(
        [np.asarray(inputs[f"{ln}_w"], f32) for ln in
         ("ln1", "ln2", "ln3")]).reshape(1, 3 * D)
    shared["w1"] = np.ascontiguousarray(
        np.asarray(inputs["ff_w1"], f32).T.reshape(NKC, P, DFF))
    shared["w2"] = np.ascontiguousarray(
        np.asarray(inputs["ff_w2"], f32).T.reshape(NFC, P, D))
    # order must match sa_bq/sa_bk/sa_bo, ca_bq/ca_bk/ca_bo above
    sm_fixed = [sm_cols[0], sm_cols[1], sm_cols[2],
                sm_cols[3], sm_cols[4], sm_cols[5],
                np.asarray(inputs["ff_b1"], f32).reshape(NFC, P).T,
                np.asarray(inputs["ff_b2"], f32).reshape(NKC, P).T]
    for ln in ("ln1", "ln2", "ln3"):
        sm_fixed.append(np.asarray(inputs[f"{ln}_b"],
                                   f32).reshape(NKC, P).T)


    maps = []
    for c in range(8):
        b, half = c // 2, c % 2
        q0 = half * OWN
        order = np.concatenate(
            [np.arange(q0, q0 + OWN), np.r_[0:q0, q0 + OWN:T]]).astype(
                np.int64)
        m = dict(shared)
        m["xT"] = np.ascontiguousarray(x[b][order].T)
        m["memT"] = np.ascontiguousarray(mem[b].T)
        mT = mem[b].T  # [D, T]
        m["memT8"] = np.ascontiguousarray(
            mT.reshape(NKC // 2, 2, P, T).transpose(0, 2, 1, 3)).astype(f8)
        mt = np.ascontiguousarray(add_tgt[b][q0:q0 + OWN][:, order].T)
        m["maskT"] = mt
        smask_c = add_src[b].reshape(NSC, P).T
        tailb_c = mt[OWN:, 0].reshape(4, P).T
        m["smalls"] = np.ascontiguousarray(
            np.concatenate(sm_fixed + [smask_c, tailb_c], axis=1))
        maps.append(m)
    return maps


def _tail_rows_constant(maps):
    """True when every core's mask s-chunks 4..7 are constant per s-row, so
    they can be applied as a per-partition exp bias instead of tensor adds."""
    for m in maps:
        tail = m["maskT"][OWN:, :]
        if not np.all(tail == tail[:, :1]):
            return False
    return True


def gather(results):
    out = np.zeros((B, T, D), np.float32)
    for c in range(8):
        b, half = c // 2, c % 2
        out[b, half * OWN:(half + 1) * OWN, :] = results[c]["outT"].T
    return out


_NC_CACHE = {}


def kernel(**inputs):
    in_maps = host_prep(inputs)
    mode = "split" if _tail_rows_constant(in_maps) else "full"
    if mode not in _NC_CACHE:
        _NC_CACHE[mode] = build_program(mask_mode=mode)
    nc = _NC_CACHE[mode]
    res = run_bass_kernel_spmd(nc, in_maps, list(range(8)))
    return gather(res.results)


if __name__ == "__main__":
    import reference as ref_mod
    inputs = {k: np.asarray(v) for k, v in ref_mod.setup_inputs().items()}
    expected = np.asarray(ref_mod.reference(**ref_mod.setup_inputs()))
    actual = kernel(**inputs)
    err = np.abs(actual - expected).max()
    rel = err / np.abs(expected).max()
    print("max abs err:", err, "rel:", rel)



# revision 26
# speedup vs baseline: 2.3299x; 1.0123x over previous
"""Transformer decoder layer (pre-norm, self-attn + cross-attn + FFN) on 8
Trainium2 NeuronCores.

Sharding: core c handles batch b = c//2 and the contiguous half of the 1024
target tokens h = c%2 (512 query rows each). K/V work is duplicated within
each batch pair so there are no collectives; every core runs an identical
program on different data. The host rotates the token order per core so that
each core's own tokens are always columns [0, 512) -> one uniform SPMD
program.

On-device layout is feature-major ([d_model, token]) throughout, so no
on-device transposes are needed: the host pre-transposes x / memory / weights
and transposes the output back. Masks are converted to additive fp32 masks on
the host.

Precision/speed: the Q/K/V projections of both attentions run as fp8e4m3
DoubleRow matmuls (x32-scaled weights, contraction k-chunk pairs packed on
tile dim 1; 2 rows/PE-cycle = 4x the f32r rate), with the scale folded into
the f32 PSUM epilogue.  LN outputs (h1/h2) and memory are quantized to fp8
for those projections only; scores, attention-value, output projections, FFN
and the residual stream stay float32r/f32 (measured end-to-end max-rel error
1.05e-2 vs the fp32 reference, tolerance 2e-2).  The attention softmax is
software-pipelined: scores for step i+1 issue on the PE before the AV
matmuls of step i so the DVE/Act exp latency stays off the critical path;
exp feeds an augmented-V matmul whose ones-column accumulates the softmax
denominator, applied via a PE rank-1 broadcast of the reciprocal.  The
cross-attention K/V projections issue between LN2's sum and normalize passes
to hide the LN stats chain.
"""

import numpy as np
from contextlib import ExitStack

import concourse.bass as bass
import concourse.bacc as bacc
import concourse.tile as tile
from concourse import mybir
from concourse.bass_utils import run_bass_kernel_spmd

D = 1024        # d_model
H = 16          # heads
DK = 64         # head dim
DFF = 4096
B = 4
T = 1024        # tgt/src len
OWN = 512       # query rows per core
P = 128         # partitions
NKC = D // P    # 8 feature chunks
NSC = T // P    # 8 s-chunks
NFC = DFF // P  # 32 ffn chunks
EPS = 1e-6

F32 = mybir.dt.float32
F32R = mybir.dt.float32r
F8 = mybir.dt.float8e4
DR = mybir.MatmulPerfMode.DoubleRow
AF = mybir.ActivationFunctionType
ALU = mybir.AluOpType


def _r(ap):
    return ap.bitcast(F32R)


# ---------------------------------------------------------------------------
# program builder (identical for every core; only DRAM contents differ)
# ---------------------------------------------------------------------------

def build_program(repeat=1, mask_mode="split"):
    nc = bacc.Bacc(None)
    dr = {}

    def din(name, shape, dt=F32):
        dr[name] = nc.dram_tensor(name, list(shape), dt, kind="ExternalInput")
        return dr[name]

    din("ones_r", [P, P], F32R)
    din("xT", [D, T], F32R)                # batch-b x, transposed, own tokens first
    din("memT", [D, T], F32R)              # memory[b] transposed
    din("maskT", [T, OWN])           # additive tgt mask, [s_rot, q_local]
    din("lnwrows", [1, 3 * D], F32R)  # LN w rows, concat (PE bcast)
    # all small per-partition tensors packed into one DMA:
    # cols: [sa_bq 8][sa_bk 8][sa_bo 8][ca_bq 8][ca_bk 8][ca_bo 8]
    #       [b1 32][b2 8][ln1_b 8][ln2_b 8][ln3_b 8][smask 8][tailb 4]
    din("smalls", [P, 124])
    for pre in ("sa", "ca"):
        din(f"{pre}_wq", [NKC // 2, P, 2, D], F8)   # x32 scaled fp8, k-pairs
        din(f"{pre}_wk", [NKC // 2, P, 2, D], F8)
        din(f"{pre}_wv", [NKC // 2, P, 2, D], F8)
        din(f"{pre}_wo", [NKC, P, D], F32R)
        din(f"{pre}_bv", [D], F32R)      # placed at partition 64 (outer trick)
    din("memT8", [NKC // 2, P, 2, T], F8)  # fp8 memory, k-pairs
    din("w1", [NKC, P, DFF], F32R)
    din("w2", [NFC, P, D], F32R)

    outT = nc.dram_tensor("outT", [D, OWN], F32, kind="ExternalOutput")

    with ExitStack() as ctx:
        tc = ctx.enter_context(tile.TileContext(nc))
        ctx.enter_context(nc.allow_low_precision(
            reason="float32r staging for full-rate PE matmuls"))
        persist = ctx.enter_context(tc.tile_pool(name="persist", bufs=1))

        ones = persist.tile([P, P], F32R, tag="ones", name="ones")
        nc.gpsimd.dma_start(ones[:], dr["ones_r"][:])
        ones_f = persist.tile([P, 1], F32, tag="ones_f", name="ones_f")
        nc.vector.memset(ones_f[:], 1.0)
        eps_11 = persist.tile([1, 1], F32, tag="eps11", name="eps11")
        nc.vector.memset(eps_11[:], EPS)

        smalls = persist.tile([P, 124], F32, tag="smalls", name="smalls")
        nc.gpsimd.dma_start(smalls[:], dr["smalls"][:])
        _off = [0]

        def s_col(n):
            t = smalls[:, _off[0]:_off[0] + n]
            _off[0] += n
            return t

        bias = {}
        for pre in ("sa", "ca"):
            for nm in ("bq", "bk", "bo"):
                bias[f"{pre}_{nm}"] = s_col(NKC)
        bias["b1"] = s_col(NFC)
        bias["b2"] = s_col(NKC)
        lnp = {}
        for ln in ("ln1", "ln2", "ln3"):
            lnp[f"{ln}_b"] = s_col(NKC)
        smask = s_col(NSC)
        tailb = s_col(4)
        for pre in ("sa", "ca"):
            bv = persist.tile([P, D], F32R, tag=f"{pre}_bv", name=f"{pre}_bv")
            nc.gpsimd.dma_start(bv[64:65, :], dr[f"{pre}_bv"][None, :])
            bias[f"{pre}_bv"] = bv
        lnwr = persist.tile([1, 3 * D], F32R, tag="lnwrows", name="lnwrows")
        nc.gpsimd.dma_start(lnwr[:], dr["lnwrows"][:])
        lnrow = {"ln1": lnwr[0:1, 0:D], "ln2": lnwr[0:1, D:2 * D],
                 "ln3": lnwr[0:1, 2 * D:3 * D]}

        # residual (own tokens), lives to the end
        xcur = [persist.tile([P, OWN], F32, tag=f"xc{i}", name=f"xc{i}")
                for i in range(NKC)]

        tmp = ctx.enter_context(tc.tile_pool(name="tmp", bufs=2))
        stats = ctx.enter_context(tc.tile_pool(name="stats", bufs=1))

        # ------------------------------------------------------------------
        def ln_sums(blk, src_get, ps_st, nbuf=1):
            """Pass 1: PE ones-reductions for sum and sum-of-squares."""
            sb0 = src_get(blk, 0)
            src_is_r = sb0.dtype == F32R
            ones_s = ones[:, 0:1] if src_is_r else ones_f[:]
            ps_s = ps_st.tile([1, 512], F32, tag="lns", name="lns", bufs=nbuf)
            ps_q = ps_st.tile([1, 512], F32, tag="lnq", name="lnq", bufs=nbuf)
            for kc in range(NKC):
                sbk = sb0 if kc == 0 else src_get(blk, kc)
                nc.tensor.matmul(ps_s[:], ones_s, sbk,
                                 start=(kc == 0), stop=(kc == NKC - 1))
                sq = tmp.tile([P, 512], F32R, tag="lnsq", name="lnsq", bufs=1)
                nc.scalar.activation(sq[:], sbk, AF.Square)
                nc.tensor.matmul(ps_q[:], ones[:, 0:1], sq[:],
                                 start=(kc == 0), stop=(kc == NKC - 1))
            return ps_s, ps_q

        def ln_normalize(blk, src_get, db, wrow, b_pn, ps_st, ps_sq):
            """Pass 2: stats chain (DVE/Act) + PE broadcasts + apply."""
            ps_s, ps_q = ps_sq
            s2 = stats.tile([1, 512], F32, tag="lnstA", name="lnstA")
            # s2 = (sum/sqrt(D*(D-1)))^2 = sum^2/(D*(D-1))
            nc.scalar.activation(s2[:], ps_s[:], AF.Square,
                                 scale=float(1.0 / np.sqrt(D * (D - 1.0))))
            var = stats.tile([1, 512], F32, tag="lnstB", name="lnstB")
            nc.vector.scalar_tensor_tensor(
                var[:], ps_q[:], 1.0 / (D - 1.0), s2[:],
                op0=ALU.mult, op1=ALU.subtract)
            std = stats.tile([1, 512], F32, tag="lnstA", name="lnstA")
            nc.scalar.activation(std[:], var[:], AF.Sqrt)
            nc.scalar.add(std[:], std[:], eps_11[:])
            rstd = stats.tile([1, 512], F32R, tag="lnstC", name="lnstC")
            nc.vector.reciprocal(rstd[:], std[:])
            mr = stats.tile([1, 512], F32R, tag="lnstB", name="lnstB")
            nc.vector.scalar_tensor_tensor(
                mr[:], ps_s[:], 1.0 / D, rstd[:],
                op0=ALU.mult, op1=ALU.mult)
            for kc in range(NKC):
                wl = wrow[0:1, kc * P:(kc + 1) * P]
                ps_rb = ps_st.tile([P, 512], F32, tag="ln_rb",
                                   name="ln_rb", bufs=1)
                nc.tensor.matmul(ps_rb[:], wl, rstd[:],
                                 start=True, stop=True)
                ps_mb = ps_st.tile([P, 512], F32, tag="ln_mb",
                                   name="ln_mb", bufs=1)
                nc.tensor.matmul(ps_mb[:], wl, mr[:],
                                 start=True, stop=True)
                t = tmp.tile([P, 512], F32, tag="lnt", name="lnt")
                nc.vector.tensor_mul(t[:], src_get(blk, kc), ps_rb[:])
                nc.vector.scalar_tensor_tensor(
                    db[kc], t[:], b_pn[:, kc:kc + 1], ps_mb[:],
                    op0=ALU.add, op1=ALU.subtract)

        def layer_norm(nblk, src_get, dst_blocks, wrow, b_pn, ps_st,
                       mid=None):
            """Feature-major LN, h = (x - mean) * (w * rstd) + b.
            All blocks' sum passes issue first (so block b+1's PE sums hide
            block b's DVE stats chain), then `mid()` may issue extra PE work,
            then the normalize passes run."""
            if mid is None:
                for blk in range(nblk):
                    s = ln_sums(blk, src_get, ps_st)
                    ln_normalize(blk, src_get, dst_blocks[blk], wrow, b_pn,
                                 ps_st, s)
            else:
                sums = [ln_sums(blk, src_get, ps_st, nbuf=nblk)
                        for blk in range(nblk)]
                mid()
                for blk in range(nblk):
                    ln_normalize(blk, src_get, dst_blocks[blk], wrow, b_pn,
                                 ps_st, sums[blk])

        # ------------------------------------------------------------------
        def load_w_rows(wpool, wname, n=NKC, cols=D):
            tiles = []
            for kc in range(n):
                wt = wpool.tile([P, cols], F32R, tag="w", name="w")
                nc.sync.dma_start(wt[:], dr[wname][kc])
                tiles.append(wt)
            return tiles

        def load_w_pairs(wpool, wname, n=NKC // 2, cols=D):
            """fp8 weight k-pair tiles [P, 2, cols] for DoubleRow."""
            tiles = []
            for kp in range(n):
                wt = wpool.tile([P, 2, cols], F8, tag="w8", name="w8")
                nc.sync.dma_start(wt[:], dr[wname][kp])
                tiles.append(wt)
            return tiles

        def proj_fm8(wpool, wname, bias_pn, src_blocks, dst, ps_acc, scl):
            """fp8 DoubleRow feature-major projection.
            dst[c][:, blk*512:..] = scl * sum_kp W8[kp,c].T @ src8[blk][kp] + b
            src8 blocks are lists of [P, 2, 512] fp8 pair tiles."""
            nblk = len(src_blocks)
            w_tiles = load_w_pairs(wpool, wname)
            for c in range(NKC):
                pss = [ps_acc.tile([P, 512], F32, tag=f"proj{blk}",
                                   name=f"proj{blk}") for blk in range(nblk)]
                for kp in range(NKC // 2):
                    for blk in range(nblk):
                        nc.tensor.matmul(pss[blk][:],
                                         w_tiles[kp][:, :, c * P:(c + 1) * P],
                                         src_blocks[blk][kp][:],
                                         start=(kp == 0),
                                         stop=(kp == NKC // 2 - 1),
                                         perf_mode=DR)
                for blk in range(nblk):
                    nc.vector.tensor_scalar(
                        dst[c][:, blk * 512:(blk + 1) * 512],
                        pss[blk][:], scl, bias_pn[:, c:c + 1],
                        op0=ALU.mult, op1=ALU.add)

        def proj_tm_vaug8(wpool, wname, src_blocks, vaug, ps_acc, scl):
            """fp8 DoubleRow token-major V projection into vaug tiles."""
            wv = load_w_pairs(wpool, wname)
            for dc in range(2):
                for st in range(NSC):
                    sb = src_blocks[st // 4]
                    t0 = (st % 4) * P
                    ps = ps_acc.tile([P, 512], F32, tag="proj0", name="proj0")
                    for kp in range(NKC // 2):
                        nc.tensor.matmul(
                            ps[:], sb[kp][:, :, t0:t0 + P],
                            wv[kp][:, :, dc * 512:(dc + 1) * 512],
                            start=(kp == 0), stop=(kp == NKC // 2 - 1),
                            perf_mode=DR)
                    nc.vector.tensor_scalar_mul(
                        vaug[st][:, 8 * dc:8 * (dc + 1), 0:DK],
                        ps[:].rearrange("p (h d) -> p h d", h=8), scl)

        # ------------------------------------------------------------------
        def attention(KT, QT, vaug, OT, mask_tiles, tail_pn, smask_pn,
                      bv_tile, att_pools, causal=False):
            """Software-pipelined: scores for step s+1 issue on the PE before
            the AV matmuls of step s, so the DVE/Act exp latency of step s
            hides under PE work.  PSUM: 4 score banks + 2 po + 1 rb = 7.
            causal=True: own-block s-chunk i only has unmasked queries
            q >= 128*i (host rotation puts own tokens first, ascending), so
            scores/mask-add/exp/AV shrink to that q-range -- exact, since the
            skipped exp values are 0 and i=0 initializes the full PSUM."""
            ps_sc, ps_av, ps_rb, epool = att_pools
            nmask = len(mask_tiles) if mask_tiles is not None else 0

            def q0_of(i):
                return i * P if (causal and i < nmask) else 0

            def issue_scores(c, i):
                q0 = q0_of(i)
                pair = [ps_sc.tile([P, 512], F32, tag="sc", name="sc", bufs=4),
                        ps_sc.tile([P, 512], F32, tag="sc", name="sc", bufs=4)]
                for h01 in (0, 1):
                    sl = slice(64 * h01, 64 * h01 + 64)
                    nc.tensor.matmul(
                        pair[h01][:, q0:512], KT[c][sl, i * P:(i + 1) * P],
                        QT[c][sl, q0:512], start=True, stop=True)
                return pair

            def issue_exp(i, pssc_h):
                q0 = q0_of(i)
                e = epool.tile([P, 512], F32R, tag="e", name="e")
                if i < nmask:
                    tm = epool.tile([P, 512], F32, tag="emask",
                                    name="emask", bufs=2)
                    nc.vector.tensor_add(tm[:, q0:512], pssc_h[:, q0:512],
                                         mask_tiles[i][:, q0:512])
                    nc.scalar.activation(e[:, q0:512], tm[:, q0:512], AF.Exp)
                elif tail_pn is not None:
                    nc.scalar.activation(e[:], pssc_h[:], AF.Exp,
                                         bias=tail_pn[:, i - 4:i - 3])
                elif smask_pn is not None:
                    nc.scalar.activation(e[:], pssc_h[:], AF.Exp,
                                         bias=smask_pn[:, i:i + 1])
                else:
                    nc.scalar.activation(e[:], pssc_h[:], AF.Exp)
                return e

            seq = [(c, i) for c in range(NKC) for i in range(NSC)]
            po_c = None
            pair = issue_scores(*seq[0])
            for idx, (c, i) in enumerate(seq):
                q0 = q0_of(i)
                if i == 0:
                    po_c = [ps_av.tile([DK + 1, 512], F32, tag="po",
                                       name="po", bufs=3),
                            ps_av.tile([DK + 1, 512], F32, tag="po",
                                       name="po", bufs=3)]
                es = [issue_exp(i, pair[0]), issue_exp(i, pair[1])]
                if idx + 1 < len(seq):
                    pair = issue_scores(*seq[idx + 1])
                for h01 in (0, 1):
                    nc.tensor.matmul(
                        po_c[h01][:, q0:512], vaug[i][:, 2 * c + h01, :],
                        es[h01][:, q0:512], start=(i == 0),
                        stop=(i == NSC - 1), skip_group_check=True)
                if i == NSC - 1:
                    for h01 in (0, 1):
                        h = 2 * c + h01
                        po = po_c
                        sums = epool.tile([P, 512], F32R, tag="sums",
                                          name="sums", bufs=2)
                        nc.vector.tensor_copy(sums[64:65, :],
                                              po[h01][DK:DK + 1, :])
                        # O_un += bv (x) sums (folds V bias through softmax)
                        nc.tensor.matmul(
                            po[h01][0:DK, :],
                            bv_tile[64:65, DK * h:DK * h + DK],
                            sums[64:65, :], start=False, stop=True,
                            skip_group_check=True)
                        nc.vector.reciprocal(sums[64:65, :], sums[64:65, :])
                        prb = ps_rb.tile([DK, 512], F32, tag="rb", name="rb")
                        nc.tensor.matmul(prb[:], ones[64:65, 0:DK],
                                         sums[64:65, :], start=True, stop=True)
                        # DVE can read only one PSUM operand: stage po first
                        o_un = epool.tile([DK, 512], F32, tag="o_un",
                                          name="o_un", bufs=2)
                        nc.vector.tensor_copy(o_un[:], po[h01][0:DK, :])
                        if h01 == 0:
                            nc.vector.tensor_mul(OT[c][0:DK, :], o_un[:],
                                                 prb[:])
                        else:
                            ot = epool.tile([DK, 512], F32R, tag="ot",
                                            name="ot", bufs=1)
                            nc.vector.tensor_mul(ot[:], o_un[:], prb[:])
                            nc.sync.dma_start(OT[c][DK:P, :], ot[:])

        def out_proj_residual(w_tiles, bias_pn, OT, ps_acc):
            for c2 in range(NKC):
                ps = ps_acc.tile([P, 512], F32, tag="proj0", name="proj0")
                for c in range(NKC):
                    nc.tensor.matmul(ps[:],
                                     w_tiles[c][:, c2 * P:(c2 + 1) * P],
                                     OT[c][:],
                                     start=(c == 0), stop=(c == NKC - 1))
                nc.vector.scalar_tensor_tensor(
                    xcur[c2][:], ps[:], bias_pn[:, c2:c2 + 1], xcur[c2][:],
                    op0=ALU.add, op1=ALU.add)

        def attention_block(prefix, pn, KT, QT, vaug, mask_tiles, tail_pn,
                            use_smask, scope, causal=False):
            """Runs attention + output projection + residual."""
            otp = scope.enter_context(
                tc.tile_pool(name=f"{pn}_otp", bufs=1))
            OT = [otp.tile([P, OWN], F32R, tag=f"OT{c}", name=f"OT{c}")[:]
                  for c in range(NKC)]
            # wo prefetch overlaps the attention phase
            wsp = scope.enter_context(tc.tile_pool(name=f"{pn}_wso", bufs=8))
            wo_tiles = load_w_rows(wsp, f"{prefix}_wo")
            with ExitStack() as att:
                ps_sc = att.enter_context(
                    tc.tile_pool(name=f"{pn}_psc", bufs=4, space="PSUM"))
                ps_av = att.enter_context(
                    tc.tile_pool(name=f"{pn}_pav", bufs=2, space="PSUM"))
                ps_rb = att.enter_context(
                    tc.tile_pool(name=f"{pn}_prb", bufs=1, space="PSUM"))
                epool = att.enter_context(
                    tc.tile_pool(name=f"{pn}_ep", bufs=3))
                attention(KT, QT, vaug, OT, mask_tiles, tail_pn,
                          smask if use_smask else None,
                          bias[f"{prefix}_bv"], (ps_sc, ps_av, ps_rb, epool),
                          causal=causal)
            with tc.tile_pool(name=f"{pn}_pso", bufs=4, space="PSUM") as pso:
                out_proj_residual(wo_tiles, bias[f"{prefix}_bo"], OT, pso)

        for _rep in range(repeat):
            # ==================================================================
            # phase 1+2: LN1, self-attention
            # ==================================================================
            with ExitStack() as sa:
                big = sa.enter_context(tc.tile_pool(name=f"sa_big{_rep}", bufs=1))
                QT = [big.tile([P, OWN], F32R, tag=f"QT{c}", name=f"QT{c}")[:]
                      for c in range(NKC)]
                KT = [big.tile([P, T], F32R, tag=f"KT{c}", name=f"KT{c}")[:]
                      for c in range(NKC)]
                vaug = [big.tile([P, H, DK + 1], F32R, tag=f"V{i}", name=f"V{i}")[:]
                        for i in range(NSC)]
                for i in range(NSC):
                    nc.vector.memset(vaug[i][:, :, DK].bitcast(F32), 1.0)

                with ExitStack() as ph:
                    h1p = ph.enter_context(
                        tc.tile_pool(name=f"h1{_rep}", bufs=1))
                    h1_blocks = [
                        [h1p.tile([P, 2, 512], F8, tag=f"h1_{blk}_{kp}",
                                  name=f"h1_{blk}_{kp}")[:]
                         for kp in range(NKC // 2)]
                        for blk in range(2)]
                    h1_dst = [
                        [h1_blocks[blk][kc // 2][:, kc % 2, :]
                         for kc in range(NKC)]
                        for blk in range(2)]
                    with ExitStack() as wscope:
                        # weight pool open during LN1 so the Q/K/V weight
                        # prefetch overlaps the LN chain
                        wsp = wscope.enter_context(
                            tc.tile_pool(name=f"sa_ws{_rep}", bufs=9))
                        psa = wscope.enter_context(
                            tc.tile_pool(name=f"ps_sap{_rep}", bufs=2,
                                         space="PSUM"))
                        with ExitStack() as lnscope:
                            xsp = lnscope.enter_context(
                                tc.tile_pool(name=f"xs{_rep}", bufs=3))
                            ps_ln = lnscope.enter_context(
                                tc.tile_pool(name=f"ps_ln1{_rep}", bufs=1,
                                             space="PSUM"))

                            def x_get(blk, kc):
                                xt = xsp.tile([P, 512], F32R, tag="xs",
                                              name="xs")
                                nc.sync.dma_start(
                                    xt[:],
                                    dr["xT"][kc * P:(kc + 1) * P,
                                             blk * 512:(blk + 1) * 512])
                                return xt[:]

                            layer_norm(2, x_get, h1_dst, lnrow["ln1"],
                                       lnp["ln1_b"], ps_ln)
                        proj_fm8(wsp, "sa_wq", bias["sa_bq"], [h1_blocks[0]],
                                 QT, psa, 1.0 / 256.0)
                        proj_fm8(wsp, "sa_wk", bias["sa_bk"], h1_blocks,
                                 KT, psa, 1.0 / 32.0)
                        proj_tm_vaug8(wsp, "sa_wv", h1_blocks, vaug, psa,
                                      1.0 / 32.0)

                # residual base (own half of x)
                for kc in range(NKC):
                    nc.sync.dma_start(
                        xcur[kc][:],
                        dr["xT"][kc * P:(kc + 1) * P, 0:OWN].bitcast(F32))

                with ExitStack() as mscope:
                    mp = mscope.enter_context(
                        tc.tile_pool(name=f"maskp{_rep}", bufs=1))
                    nmt = 4 if mask_mode == "split" else NSC
                    mask_tiles = []
                    for i in range(nmt):
                        mt = mp.tile([P, OWN], F32, tag=f"mask{i}",
                                     name=f"mask{i}")
                        nc.sync.dma_start(mt[:],
                                          dr["maskT"][i * P:(i + 1) * P, :])
                        mask_tiles.append(mt[:])
                    attention_block(
                        "sa", f"sa{_rep}", KT, QT, vaug, mask_tiles,
                        tailb if mask_mode == "split" else None,
                        False, mscope, causal=(mask_mode == "split"))

            # ==================================================================
            # phase 3: cross attention
            # ==================================================================
            with ExitStack() as ca:
                big = ca.enter_context(tc.tile_pool(name=f"ca_big{_rep}", bufs=1))
                QT = [big.tile([P, OWN], F32R, tag=f"cQT{c}", name=f"cQT{c}")[:]
                      for c in range(NKC)]
                KT = [big.tile([P, T], F32R, tag=f"cKT{c}", name=f"cKT{c}")[:]
                      for c in range(NKC)]
                vaug = [big.tile([P, H, DK + 1], F32R, tag=f"cV{i}",
                                 name=f"cV{i}")[:] for i in range(NSC)]
                for i in range(NSC):
                    nc.vector.memset(vaug[i][:, :, DK].bitcast(F32), 1.0)

                # K/V projections first: they depend only on memory, so
                # they keep the PE busy while the LN2 chain runs.
                with ExitStack() as mm_scope:
                    wsp = mm_scope.enter_context(
                        tc.tile_pool(name=f"ca_wkv{_rep}", bufs=9))
                    pca = mm_scope.enter_context(
                        tc.tile_pool(name=f"ps_ckv{_rep}", bufs=2,
                                     space="PSUM"))
                    with ExitStack() as mscope2:
                        memp = mscope2.enter_context(
                            tc.tile_pool(name=f"mem{_rep}", bufs=1))
                        mem_blocks = []
                        for blk in range(2):
                            mb = []
                            for kp in range(NKC // 2):
                                mt = memp.tile([P, 2, 512], F8,
                                               tag=f"m{blk}_{kp}",
                                               name=f"m{blk}_{kp}")
                                nc.sync.dma_start(
                                    mt[:],
                                    dr["memT8"][kp][:, :,
                                                    blk * 512:(blk + 1) * 512])
                                mb.append(mt[:])
                            mem_blocks.append(mb)

                        with ExitStack() as ph:
                            h2p = ph.enter_context(
                                tc.tile_pool(name=f"h2{_rep}", bufs=1))
                            h2 = [h2p.tile([P, 2, OWN], F8, tag=f"h2_{kp}",
                                           name=f"h2_{kp}")[:]
                                  for kp in range(NKC // 2)]
                            h2_dst = [h2[kc // 2][:, kc % 2, :]
                                      for kc in range(NKC)]

                            def ca_kv_mid():
                                proj_fm8(wsp, "ca_wk", bias["ca_bk"],
                                         mem_blocks, KT, pca, 1.0 / 32.0)
                                proj_tm_vaug8(wsp, "ca_wv", mem_blocks,
                                              vaug, pca, 1.0 / 32.0)

                            with tc.tile_pool(name=f"ps_ln2{_rep}", bufs=1,
                                              space="PSUM") as psl:
                                layer_norm(1, lambda blk, kc: xcur[kc][:],
                                           [h2_dst], lnrow["ln2"],
                                           lnp["ln2_b"], psl, mid=ca_kv_mid)
                            proj_fm8(wsp, "ca_wq", bias["ca_bq"], [h2], QT,
                                     pca, 1.0 / 256.0)

                with ExitStack() as ascope:
                    attention_block("ca", f"ca{_rep}", KT, QT, vaug, None,
                                None, True, ascope)

            # ==================================================================
            # phase 4: FFN
            # ==================================================================
            with ExitStack() as ff:
                ap_pool = ff.enter_context(tc.tile_pool(name=f"aT{_rep}", bufs=1))
                w2p = ff.enter_context(tc.tile_pool(name=f"w2p{_rep}", bufs=2))
                aT = [ap_pool.tile([P, OWN], F32R, tag=f"aT{i}", name=f"aT{i}")[:]
                      for i in range(NFC)]
                with ExitStack() as ph:
                    h3p = ph.enter_context(
                        tc.tile_pool(name=f"h3{_rep}", bufs=1))
                    h3 = [h3p.tile([P, OWN], F32R, tag=f"h3_{kc}",
                                   name=f"h3_{kc}")[:] for kc in range(NKC)]
                    with ExitStack() as wscope:
                        wsp = wscope.enter_context(
                            tc.tile_pool(name=f"ff_ws{_rep}", bufs=10))
                        with tc.tile_pool(name=f"ps_ln3{_rep}", bufs=1,
                                          space="PSUM") as psl:
                            layer_norm(1, lambda blk, kc: xcur[kc][:], [h3],
                                       lnrow["ln3"], lnp["ln3_b"], psl)
                        ps_f1 = wscope.enter_context(
                            tc.tile_pool(name=f"ps_ff1{_rep}", bufs=1,
                                         space="PSUM"))
                        for cg in range(4):
                            w1t = []
                            for kc in range(NKC):
                                wt = wsp.tile([P, 1024], F32R, tag="w",
                                              name="w")
                                nc.sync.dma_start(
                                    wt[:], dr["w1"][kc][:, cg * 1024:
                                                        (cg + 1) * 1024])
                                w1t.append(wt)
                            for cc in range(NKC):
                                cidx = cg * 8 + cc
                                ps = ps_f1.tile([P, 512], F32, tag="proj0",
                                                name="proj0", bufs=4)
                                for kc in range(NKC):
                                    nc.tensor.matmul(
                                        ps[:], w1t[kc][:, cc * P:(cc + 1) * P],
                                        h3[kc], start=(kc == 0),
                                        stop=(kc == NKC - 1))
                                nc.scalar.activation(
                                    aT[cidx], ps[:], AF.Relu,
                                    bias=bias["b1"][:, cidx:cidx + 1])

                with ExitStack() as yscope:
                    ps_y = yscope.enter_context(
                        tc.tile_pool(name=f"ps_y{_rep}", bufs=1, space="PSUM"))
                    yps = [ps_y.tile([P, 512], F32, tag=f"y{c2}", name=f"y{c2}")
                           for c2 in range(NKC)]
                    for kc2 in range(NFC):
                        wt = w2p.tile([P, D], F32R, tag="w", name="w",
                                      bufs=6)
                        nc.sync.dma_start(wt[:], dr["w2"][kc2])
                        for c2 in range(NKC):
                            nc.tensor.matmul(
                                yps[c2][:], wt[:, c2 * P:(c2 + 1) * P],
                                aT[kc2], start=(kc2 == 0),
                                stop=(kc2 == NFC - 1))
                    for c2 in range(NKC):
                        nc.vector.scalar_tensor_tensor(
                            xcur[c2][:], yps[c2][:], bias["b2"][:, c2:c2 + 1],
                            xcur[c2][:], op0=ALU.add, op1=ALU.add)

            for c2 in range(NKC):
                nc.sync.dma_start(outT[c2 * P:(c2 + 1) * P, :], xcur[c2][:])

    nc.finalize()
    return nc


# ---------------------------------------------------------------------------
# host side
# ---------------------------------------------------------------------------

def _tile4(wT):
    """[D_in, D_out] -> [D_in/P, D_out/P, P, P] contiguous tiles."""
    di, do = wT.shape
    return np.ascontiguousarray(
        wT.reshape(di // P, P, do // P, P).transpose(0, 2, 1, 3))


def host_prep(inputs):
    f32 = np.float32
    x = np.asarray(inputs["x"], f32)
    mem = np.asarray(inputs["memory"], f32)
    tgt = np.asarray(inputs["tgt_mask"])
    src = np.asarray(inputs["src_mask"])

    add_tgt = (tgt.astype(f32) - 1.0) * 1e9     # [B, T, T]: 0 or -1e9
    add_src = (src.astype(f32) - 1.0) * 1e9     # [B, T]

    import ml_dtypes
    f8 = ml_dtypes.float8_e4m3

    def pack8(wT):
        # [D, cols] -> fp8 x32 k-pair tiles [NKC//2, P, 2, cols]
        d, cols = wT.shape
        return np.ascontiguousarray(
            (wT * 32.0).reshape(d // P // 2, 2, P, cols)
            .transpose(0, 2, 1, 3)).astype(f8)

    shared = {"ones_r": np.ones((P, P), f32)}
    sm_cols = []
    for pre in ("sa", "ca"):
        wq = np.asarray(inputs[f"{pre}_wq"], f32)
        shared[f"{pre}_wq"] = pack8(wq.T)
        shared[f"{pre}_wk"] = pack8(np.asarray(inputs[f"{pre}_wk"], f32).T)
        shared[f"{pre}_wv"] = pack8(np.asarray(inputs[f"{pre}_wv"], f32).T)
        shared[f"{pre}_wo"] = np.ascontiguousarray(
            np.asarray(inputs[f"{pre}_wo"], f32).T.reshape(NKC, P, D))
        shared[f"{pre}_bv"] = np.asarray(inputs[f"{pre}_bv"], f32)
    for pre in ("sa", "ca"):
        sm_cols.append((np.asarray(inputs[f"{pre}_bq"], f32)
                        * 0.125).reshape(NKC, P).T)
        sm_cols.append(np.asarray(inputs[f"{pre}_bk"],
                                  f32).reshape(NKC, P).T)
        sm_cols.append(np.asarray(inputs[f"{pre}_bo"],
                                  f32).reshape(NKC, P).T)
    shared["lnwrows"] = np.concatenate(
        [np.asarray(inputs[f"{ln}_w"], f32) for ln in
         ("ln1", "ln2", "ln3")]).reshape(1, 3 * D)
    shared["w1"] = np.ascontiguousarray(
        np.asarray(inputs["ff_w1"], f32).T.reshape(NKC, P, DFF))
    shared["w2"] = np.ascontiguousarray(
        np.asarray(inputs["ff_w2"], f32).T.reshape(NFC, P, D))
    # order must match sa_bq/sa_bk/sa_bo, ca_bq/ca_bk/ca_bo above
    sm_fixed = [sm_cols[0], sm_cols[1], sm_cols[2],
                sm_cols[3], sm_cols[4], sm_cols[5],
                np.asarray(inputs["ff_b1"], f32).reshape(NFC, P).T,
                np.asarray(inputs["ff_b2"], f32).reshape(NKC, P).T]
    for ln in ("ln1", "ln2", "ln3"):
        sm_fixed.append(np.asarray(inputs[f"{ln}_b"],
                                   f32).reshape(NKC, P).T)


    maps = []
    for c in range(8):
        b, half = c // 2, c % 2
        q0 = half * OWN
        order = np.concatenate(
            [np.arange(q0, q0 + OWN), np.r_[0:q0, q0 + OWN:T]]).astype(
                np.int64)
        m = dict(shared)
        m["xT"] = np.ascontiguousarray(x[b][order].T)
        m["memT"] = np.ascontiguousarray(mem[b].T)
        mT = mem[b].T  # [D, T]
        m["memT8"] = np.ascontiguousarray(
            mT.reshape(NKC // 2, 2, P, T).transpose(0, 2, 1, 3)).astype(f8)
        mt = np.ascontiguousarray(add_tgt[b][q0:q0 + OWN][:, order].T)
        m["maskT"] = mt
        smask_c = add_src[b].reshape(NSC, P).T
        tailb_c = mt[OWN:, 0].reshape(4, P).T
        m["smalls"] = np.ascontiguousarray(
            np.concatenate(sm_fixed + [smask_c, tailb_c], axis=1))
        maps.append(m)
    return maps


def _tail_rows_constant(maps):
    """True when every core's mask s-chunks 4..7 are constant per s-row, so
    they can be applied as a per-partition exp bias instead of tensor adds."""
    for m in maps:
        tail = m["maskT"][OWN:, :]
        if not np.all(tail == tail[:, :1]):
            return False
    return True


def gather(results):
    out = np.zeros((B, T, D), np.float32)
    for c in range(8):
        b, half = c // 2, c % 2
        out[b, half * OWN:(half + 1) * OWN, :] = results[c]["outT"].T
    return out


_NC_CACHE = {}


def kernel(**inputs):
    in_maps = host_prep(inputs)
    mode = "split" if _tail_rows_constant(in_maps) else "full"
    if mode not in _NC_CACHE:
        _NC_CACHE[mode] = build_program(mask_mode=mode)
    nc = _NC_CACHE[mode]
    res = run_bass_kernel_spmd(nc, in_maps, list(range(8)))
    return gather(res.results)


if __name__ == "__main__":
    import reference as ref_mod
    inputs = {k: np.asarray(v) for k, v in ref_mod.setup_inputs().items()}
    expected = np.asarray(ref_mod.reference(**ref_mod.setup_inputs()))
    actual = kernel(**inputs)
    err = np.abs(actual - expected).max()
    rel = err / np.abs(expected).max()
    print("max abs err:", err, "rel:", rel)



# revision 33
# speedup vs baseline: 2.3536x; 1.0102x over previous
"""Transformer decoder layer (pre-norm, self-attn + cross-attn + FFN) on 8
Trainium2 NeuronCores.

Sharding: core c handles batch b = c//2 and the contiguous half of the 1024
target tokens h = c%2 (512 query rows each). K/V work is duplicated within
each batch pair so there are no collectives; every core runs an identical
program on different data. The host rotates the token order per core so that
each core's own tokens are always columns [0, 512) -> one uniform SPMD
program.

On-device layout is feature-major ([d_model, token]) throughout, so no
on-device transposes are needed: the host pre-transposes x / memory / weights
and transposes the output back. Masks are converted to additive fp32 masks on
the host.

Precision/speed: the Q/K/V projections of both attentions run as fp8e4m3
DoubleRow matmuls (x32-scaled weights, contraction k-chunk pairs packed on
tile dim 1; 2 rows/PE-cycle = 4x the f32r rate), with the scale folded into
the f32 PSUM epilogue.  LN outputs (h1/h2) and memory are quantized to fp8
for those projections only; scores, attention-value, output projections, FFN
and the residual stream stay float32r/f32 (measured end-to-end max-rel error
1.05e-2 vs the fp32 reference, tolerance 2e-2).  The attention softmax is
software-pipelined: scores for step i+1 issue on the PE before the AV
matmuls of step i so the DVE/Act exp latency stays off the critical path;
exp feeds an augmented-V matmul whose ones-column accumulates the softmax
denominator, applied via a PE rank-1 broadcast of the reciprocal.  The
cross-attention K/V projections issue between LN2's sum and normalize passes
to hide the LN stats chain.
"""

import numpy as np
from contextlib import ExitStack

import concourse.bass as bass
import concourse.bacc as bacc
import concourse.tile as tile
from concourse import mybir
from concourse.bass_utils import run_bass_kernel_spmd

D = 1024        # d_model
H = 16          # heads
DK = 64         # head dim
DFF = 4096
B = 4
T = 1024        # tgt/src len
OWN = 512       # query rows per core
P = 128         # partitions
NKC = D // P    # 8 feature chunks
NSC = T // P    # 8 s-chunks
NFC = DFF // P  # 32 ffn chunks
EPS = 1e-6

F32 = mybir.dt.float32
F32R = mybir.dt.float32r
F8 = mybir.dt.float8e4
DR = mybir.MatmulPerfMode.DoubleRow
AF = mybir.ActivationFunctionType
ALU = mybir.AluOpType


def _r(ap):
    return ap.bitcast(F32R)


# ---------------------------------------------------------------------------
# program builder (identical for every core; only DRAM contents differ)
# ---------------------------------------------------------------------------

def build_program(repeat=1, mask_mode="split"):
    nc = bacc.Bacc(None)
    dr = {}

    def din(name, shape, dt=F32):
        dr[name] = nc.dram_tensor(name, list(shape), dt, kind="ExternalInput")
        return dr[name]

    din("ones_r", [P, P], F32R)
    din("xT", [D, T], F32R)                # batch-b x, transposed, own tokens first
    din("memT", [D, T], F32R)              # memory[b] transposed
    din("maskT", [T, OWN])           # additive tgt mask, [s_rot, q_local]
    din("lnwrows", [1, 3 * D], F32R)  # LN w rows, concat (PE bcast)
    # all small per-partition tensors packed into one DMA:
    # cols: [sa_bq 8][sa_bk 8][sa_bo 8][ca_bq 8][ca_bk 8][ca_bo 8]
    #       [b1 32][b2 8][ln1_b 8][ln2_b 8][ln3_b 8][smask 8][tailb 4]
    din("smalls", [P, 124])
    for pre in ("sa", "ca"):
        din(f"{pre}_wq", [NKC // 2, P, 2, D], F8)   # x32 scaled fp8, k-pairs
        din(f"{pre}_wk", [NKC // 2, P, 2, D], F8)
        din(f"{pre}_wv", [NKC // 2, P, 2, D], F8)
        din(f"{pre}_wo", [NKC, P, D], F32R)
        din(f"{pre}_bv", [D], F32R)      # placed at partition 64 (outer trick)
    din("memT8", [NKC // 2, P, 2, T], F8)  # fp8 memory, k-pairs
    din("w1", [NKC, P, DFF], F32R)
    din("w2", [NFC, P, D], F32R)

    outT = nc.dram_tensor("outT", [D, OWN], F32, kind="ExternalOutput")

    with ExitStack() as ctx:
        tc = ctx.enter_context(tile.TileContext(nc))
        ctx.enter_context(nc.allow_low_precision(
            reason="float32r staging for full-rate PE matmuls"))
        persist = ctx.enter_context(tc.tile_pool(name="persist", bufs=1))

        ones = persist.tile([P, P], F32R, tag="ones", name="ones")
        nc.gpsimd.dma_start(ones[:], dr["ones_r"][:])
        ones_f = persist.tile([P, 1], F32, tag="ones_f", name="ones_f")
        nc.vector.memset(ones_f[:], 1.0)
        eps_11 = persist.tile([1, 1], F32, tag="eps11", name="eps11")
        nc.vector.memset(eps_11[:], EPS)

        smalls = persist.tile([P, 124], F32, tag="smalls", name="smalls")
        nc.gpsimd.dma_start(smalls[:], dr["smalls"][:])
        _off = [0]

        def s_col(n):
            t = smalls[:, _off[0]:_off[0] + n]
            _off[0] += n
            return t

        bias = {}
        for pre in ("sa", "ca"):
            for nm in ("bq", "bk", "bo"):
                bias[f"{pre}_{nm}"] = s_col(NKC)
        bias["b1"] = s_col(NFC)
        bias["b2"] = s_col(NKC)
        lnp = {}
        for ln in ("ln1", "ln2", "ln3"):
            lnp[f"{ln}_b"] = s_col(NKC)
        smask = s_col(NSC)
        tailb = s_col(4)
        for pre in ("sa", "ca"):
            bv = persist.tile([P, D], F32R, tag=f"{pre}_bv", name=f"{pre}_bv")
            nc.gpsimd.dma_start(bv[64:65, :], dr[f"{pre}_bv"][None, :])
            bias[f"{pre}_bv"] = bv
        lnwr = persist.tile([1, 3 * D], F32R, tag="lnwrows", name="lnwrows")
        nc.gpsimd.dma_start(lnwr[:], dr["lnwrows"][:])
        lnrow = {"ln1": lnwr[0:1, 0:D], "ln2": lnwr[0:1, D:2 * D],
                 "ln3": lnwr[0:1, 2 * D:3 * D]}

        # residual (own tokens), lives to the end
        xcur = [persist.tile([P, OWN], F32, tag=f"xc{i}", name=f"xc{i}")
                for i in range(NKC)]

        tmp = ctx.enter_context(tc.tile_pool(name="tmp", bufs=2))
        stats = ctx.enter_context(tc.tile_pool(name="stats", bufs=1))

        # ------------------------------------------------------------------
        def ln_sums(blk, src_get, ps_st, nbuf=1):
            """Pass 1: PE ones-reductions for sum and sum-of-squares."""
            sb0 = src_get(blk, 0)
            src_is_r = sb0.dtype == F32R
            ones_s = ones[:, 0:1] if src_is_r else ones_f[:]
            ps_s = ps_st.tile([1, 512], F32, tag="lns", name="lns", bufs=nbuf)
            ps_q = ps_st.tile([1, 512], F32, tag="lnq", name="lnq", bufs=nbuf)
            for kc in range(NKC):
                sbk = sb0 if kc == 0 else src_get(blk, kc)
                nc.tensor.matmul(ps_s[:], ones_s, sbk,
                                 start=(kc == 0), stop=(kc == NKC - 1))
                sq = tmp.tile([P, 512], F32R, tag="lnsq", name="lnsq", bufs=1)
                nc.scalar.activation(sq[:], sbk, AF.Square)
                nc.tensor.matmul(ps_q[:], ones[:, 0:1], sq[:],
                                 start=(kc == 0), stop=(kc == NKC - 1))
            return ps_s, ps_q

        def ln_normalize(blk, src_get, db, wrow, b_pn, ps_st, ps_sq):
            """Pass 2: stats chain (DVE/Act) + PE broadcasts + apply."""
            ps_s, ps_q = ps_sq
            s2 = stats.tile([1, 512], F32, tag="lnstA", name="lnstA")
            # s2 = (sum/sqrt(D*(D-1)))^2 = sum^2/(D*(D-1))
            nc.scalar.activation(s2[:], ps_s[:], AF.Square,
                                 scale=float(1.0 / np.sqrt(D * (D - 1.0))))
            var = stats.tile([1, 512], F32, tag="lnstB", name="lnstB")
            nc.vector.scalar_tensor_tensor(
                var[:], ps_q[:], 1.0 / (D - 1.0), s2[:],
                op0=ALU.mult, op1=ALU.subtract)
            std = stats.tile([1, 512], F32, tag="lnstA", name="lnstA")
            nc.scalar.activation(std[:], var[:], AF.Sqrt)
            nc.scalar.add(std[:], std[:], eps_11[:])
            rstd = stats.tile([1, 512], F32R, tag="lnstC", name="lnstC")
            nc.vector.reciprocal(rstd[:], std[:])
            mr = stats.tile([1, 512], F32R, tag="lnstB", name="lnstB")
            nc.vector.scalar_tensor_tensor(
                mr[:], ps_s[:], 1.0 / D, rstd[:],
                op0=ALU.mult, op1=ALU.mult)
            for kc in range(NKC):
                wl = wrow[0:1, kc * P:(kc + 1) * P]
                ps_rb = ps_st.tile([P, 512], F32, tag="ln_rb",
                                   name="ln_rb", bufs=1)
                nc.tensor.matmul(ps_rb[:], wl, rstd[:],
                                 start=True, stop=True)
                ps_mb = ps_st.tile([P, 512], F32, tag="ln_mb",
                                   name="ln_mb", bufs=1)
                nc.tensor.matmul(ps_mb[:], wl, mr[:],
                                 start=True, stop=True)
                t = tmp.tile([P, 512], F32, tag="lnt", name="lnt")
                nc.vector.tensor_mul(t[:], src_get(blk, kc), ps_rb[:])
                nc.vector.scalar_tensor_tensor(
                    db[kc], t[:], b_pn[:, kc:kc + 1], ps_mb[:],
                    op0=ALU.add, op1=ALU.subtract)

        def layer_norm(nblk, src_get, dst_blocks, wrow, b_pn, ps_st,
                       mid=None):
            """Feature-major LN, h = (x - mean) * (w * rstd) + b.
            All blocks' sum passes issue first (so block b+1's PE sums hide
            block b's DVE stats chain), then `mid()` may issue extra PE work,
            then the normalize passes run."""
            if mid is None:
                for blk in range(nblk):
                    s = ln_sums(blk, src_get, ps_st)
                    ln_normalize(blk, src_get, dst_blocks[blk], wrow, b_pn,
                                 ps_st, s)
            else:
                sums = [ln_sums(blk, src_get, ps_st, nbuf=nblk)
                        for blk in range(nblk)]
                mid()
                for blk in range(nblk):
                    ln_normalize(blk, src_get, dst_blocks[blk], wrow, b_pn,
                                 ps_st, sums[blk])

        # ------------------------------------------------------------------
        def load_w_rows(wpool, wname, n=NKC, cols=D):
            tiles = []
            for kc in range(n):
                wt = wpool.tile([P, cols], F32R, tag="w", name="w")
                nc.sync.dma_start(wt[:], dr[wname][kc])
                tiles.append(wt)
            return tiles

        def load_w_pairs(wpool, wname, n=NKC // 2, cols=D):
            """fp8 weight k-pair tiles [P, 2, cols] for DoubleRow."""
            tiles = []
            for kp in range(n):
                wt = wpool.tile([P, 2, cols], F8, tag="w8", name="w8")
                nc.sync.dma_start(wt[:], dr[wname][kp])
                tiles.append(wt)
            return tiles

        def proj_fm8(wpool, wname, bias_pn, src_blocks, dst, ps_acc, scl):
            """fp8 DoubleRow feature-major projection.
            dst[c][:, blk*512:..] = scl * sum_kp W8[kp,c].T @ src8[blk][kp] + b
            src8 blocks are lists of [P, 2, 512] fp8 pair tiles."""
            nblk = len(src_blocks)
            w_tiles = load_w_pairs(wpool, wname)
            for c in range(NKC):
                pss = [ps_acc.tile([P, 512], F32, tag=f"proj{blk}",
                                   name=f"proj{blk}") for blk in range(nblk)]
                for kp in range(NKC // 2):
                    for blk in range(nblk):
                        nc.tensor.matmul(pss[blk][:],
                                         w_tiles[kp][:, :, c * P:(c + 1) * P],
                                         src_blocks[blk][kp][:],
                                         start=(kp == 0),
                                         stop=(kp == NKC // 2 - 1),
                                         perf_mode=DR)
                for blk in range(nblk):
                    nc.vector.tensor_scalar(
                        dst[c][:, blk * 512:(blk + 1) * 512],
                        pss[blk][:], scl, bias_pn[:, c:c + 1],
                        op0=ALU.mult, op1=ALU.add)

        def proj_tm_vaug8(wpool, wname, src_blocks, vaug, ps_acc, scl):
            """fp8 DoubleRow token-major V projection into vaug tiles."""
            wv = load_w_pairs(wpool, wname)
            for dc in range(2):
                for st in range(NSC):
                    sb = src_blocks[st // 4]
                    t0 = (st % 4) * P
                    ps = ps_acc.tile([P, 512], F32, tag="proj0", name="proj0")
                    for kp in range(NKC // 2):
                        nc.tensor.matmul(
                            ps[:], sb[kp][:, :, t0:t0 + P],
                            wv[kp][:, :, dc * 512:(dc + 1) * 512],
                            start=(kp == 0), stop=(kp == NKC // 2 - 1),
                            perf_mode=DR)
                    nc.vector.tensor_scalar_mul(
                        vaug[st][:, 8 * dc:8 * (dc + 1), 0:DK],
                        ps[:].rearrange("p (h d) -> p h d", h=8), scl)

        # ------------------------------------------------------------------
        def attention(KT, QT, vaug, OT, mask_tiles, tail_pn, smask_pn,
                      bv_tile, att_pools, causal=False):
            """Software-pipelined: scores for step s+1 issue on the PE before
            the AV matmuls of step s, so the DVE/Act exp latency of step s
            hides under PE work.  PSUM: 4 score banks + 2 po + 1 rb = 7.
            causal=True: own-block s-chunk i only has unmasked queries
            q >= 128*i (host rotation puts own tokens first, ascending), so
            scores/mask-add/exp/AV shrink to that q-range -- exact, since the
            skipped exp values are 0 and i=0 initializes the full PSUM."""
            ps_sc, ps_av, ps_rb, epool = att_pools
            nmask = len(mask_tiles) if mask_tiles is not None else 0

            def q0_of(i):
                return i * P if (causal and i < nmask) else 0

            def issue_scores(c, i):
                q0 = q0_of(i)
                pair = [ps_sc.tile([P, 512], F32, tag="sc", name="sc", bufs=4),
                        ps_sc.tile([P, 512], F32, tag="sc", name="sc", bufs=4)]
                for h01 in (0, 1):
                    sl = slice(64 * h01, 64 * h01 + 64)
                    nc.tensor.matmul(
                        pair[h01][:, q0:512], KT[c][sl, i * P:(i + 1) * P],
                        QT[c][sl, q0:512], start=True, stop=True)
                return pair

            def issue_exp(i, pssc_h):
                q0 = q0_of(i)
                e = epool.tile([P, 512], F32R, tag="e", name="e")
                if i < nmask:
                    tm = epool.tile([P, 512], F32, tag="emask",
                                    name="emask", bufs=2)
                    nc.vector.tensor_add(tm[:, q0:512], pssc_h[:, q0:512],
                                         mask_tiles[i][:, q0:512])
                    nc.scalar.activation(e[:, q0:512], tm[:, q0:512], AF.Exp)
                elif tail_pn is not None:
                    nc.scalar.activation(e[:], pssc_h[:], AF.Exp,
                                         bias=tail_pn[:, i - 4:i - 3])
                elif smask_pn is not None:
                    nc.scalar.activation(e[:], pssc_h[:], AF.Exp,
                                         bias=smask_pn[:, i:i + 1])
                else:
                    nc.scalar.activation(e[:], pssc_h[:], AF.Exp)
                return e

            seq = [(c, i) for c in range(NKC) for i in range(NSC)]
            po_c = None
            pair = issue_scores(*seq[0])
            for idx, (c, i) in enumerate(seq):
                q0 = q0_of(i)
                if i == 0:
                    po_c = [ps_av.tile([DK + 1, 512], F32, tag="po",
                                       name="po", bufs=3),
                            ps_av.tile([DK + 1, 512], F32, tag="po",
                                       name="po", bufs=3)]
                es = [issue_exp(i, pair[0]), issue_exp(i, pair[1])]
                if idx + 1 < len(seq):
                    pair = issue_scores(*seq[idx + 1])
                for h01 in (0, 1):
                    nc.tensor.matmul(
                        po_c[h01][:, q0:512], vaug[i][:, 2 * c + h01, :],
                        es[h01][:, q0:512], start=(i == 0),
                        stop=(i == NSC - 1), skip_group_check=True)
                if i == NSC - 1:
                    for h01 in (0, 1):
                        heng = nc.vector
                        h = 2 * c + h01
                        po = po_c
                        sums = epool.tile([P, 512], F32R, tag="sums",
                                          name="sums", bufs=2)
                        heng.tensor_copy(sums[64:65, :],
                                         po[h01][DK:DK + 1, :])
                        # O_un += bv (x) sums (folds V bias through softmax)
                        nc.tensor.matmul(
                            po[h01][0:DK, :],
                            bv_tile[64:65, DK * h:DK * h + DK],
                            sums[64:65, :], start=False, stop=True,
                            skip_group_check=True)
                        nc.vector.reciprocal(sums[64:65, :], sums[64:65, :])
                        prb = ps_rb.tile([DK, 512], F32, tag="rb", name="rb")
                        nc.tensor.matmul(prb[:], ones[64:65, 0:DK],
                                         sums[64:65, :], start=True, stop=True)
                        # DVE can read only one PSUM operand: stage po first
                        o_un = epool.tile([DK, 512], F32, tag="o_un",
                                          name="o_un", bufs=2)
                        heng.tensor_copy(o_un[:], po[h01][0:DK, :])
                        if h01 == 0:
                            heng.tensor_mul(OT[c][0:DK, :], o_un[:],
                                            prb[:])
                        else:
                            ot = epool.tile([DK, 512], F32R, tag="ot",
                                            name="ot", bufs=1)
                            nc.vector.tensor_mul(ot[:], o_un[:], prb[:])
                            nc.sync.dma_start(OT[c][DK:P, :], ot[:])

        def out_proj_residual(w_tiles, bias_pn, OT, ps_acc):
            for c2 in range(NKC):
                ps = ps_acc.tile([P, 512], F32, tag="proj0", name="proj0")
                for c in range(NKC):
                    nc.tensor.matmul(ps[:],
                                     w_tiles[c][:, c2 * P:(c2 + 1) * P],
                                     OT[c][:],
                                     start=(c == 0), stop=(c == NKC - 1))
                nc.vector.scalar_tensor_tensor(
                    xcur[c2][:], ps[:], bias_pn[:, c2:c2 + 1], xcur[c2][:],
                    op0=ALU.add, op1=ALU.add)

        def attention_block(prefix, pn, KT, QT, vaug, mask_tiles, tail_pn,
                            use_smask, scope, causal=False):
            """Runs attention + output projection + residual."""
            otp = scope.enter_context(
                tc.tile_pool(name=f"{pn}_otp", bufs=1))
            OT = [otp.tile([P, OWN], F32R, tag=f"OT{c}", name=f"OT{c}")[:]
                  for c in range(NKC)]
            # wo prefetch overlaps the attention phase
            wsp = scope.enter_context(tc.tile_pool(name=f"{pn}_wso", bufs=8))
            wo_tiles = load_w_rows(wsp, f"{prefix}_wo")
            with ExitStack() as att:
                ps_sc = att.enter_context(
                    tc.tile_pool(name=f"{pn}_psc", bufs=4, space="PSUM"))
                ps_av = att.enter_context(
                    tc.tile_pool(name=f"{pn}_pav", bufs=2, space="PSUM"))
                ps_rb = att.enter_context(
                    tc.tile_pool(name=f"{pn}_prb", bufs=1, space="PSUM"))
                epool = att.enter_context(
                    tc.tile_pool(name=f"{pn}_ep", bufs=3))
                attention(KT, QT, vaug, OT, mask_tiles, tail_pn,
                          smask if use_smask else None,
                          bias[f"{prefix}_bv"], (ps_sc, ps_av, ps_rb, epool),
                          causal=causal)
            with tc.tile_pool(name=f"{pn}_pso", bufs=4, space="PSUM") as pso:
                out_proj_residual(wo_tiles, bias[f"{prefix}_bo"], OT, pso)

        for _rep in range(repeat):
            # ==================================================================
            # phase 1+2: LN1, self-attention
            # ==================================================================
            with ExitStack() as sa:
                big = sa.enter_context(tc.tile_pool(name=f"sa_big{_rep}", bufs=1))
                QT = [big.tile([P, OWN], F32R, tag=f"QT{c}", name=f"QT{c}")[:]
                      for c in range(NKC)]
                KT = [big.tile([P, T], F32R, tag=f"KT{c}", name=f"KT{c}")[:]
                      for c in range(NKC)]
                vaug = [big.tile([P, H, DK + 1], F32R, tag=f"V{i}", name=f"V{i}")[:]
                        for i in range(NSC)]
                for i in range(NSC):
                    nc.vector.memset(vaug[i][:, :, DK].bitcast(F32), 1.0)

                with ExitStack() as ph:
                    h1p = ph.enter_context(
                        tc.tile_pool(name=f"h1{_rep}", bufs=1))
                    h1_blocks = [
                        [h1p.tile([P, 2, 512], F8, tag=f"h1_{blk}_{kp}",
                                  name=f"h1_{blk}_{kp}")[:]
                         for kp in range(NKC // 2)]
                        for blk in range(2)]
                    h1_dst = [
                        [h1_blocks[blk][kc // 2][:, kc % 2, :]
                         for kc in range(NKC)]
                        for blk in range(2)]
                    with ExitStack() as wscope:
                        # weight pool open during LN1 so the Q/K/V weight
                        # prefetch overlaps the LN chain
                        wsp = wscope.enter_context(
                            tc.tile_pool(name=f"sa_ws{_rep}", bufs=9))
                        psa = wscope.enter_context(
                            tc.tile_pool(name=f"ps_sap{_rep}", bufs=2,
                                         space="PSUM"))
                        with ExitStack() as lnscope:
                            xsp = lnscope.enter_context(
                                tc.tile_pool(name=f"xs{_rep}", bufs=1))
                            ps_ln = lnscope.enter_context(
                                tc.tile_pool(name=f"ps_ln1{_rep}", bufs=1,
                                             space="PSUM"))
                            # x resident as 4 batched tiles: one DMA per
                            # 4 feature chunks (amortizes descriptor cost),
                            # read by both the sum and the apply pass
                            xbig = []
                            for j in range(4):
                                blk, hf = j // 2, j % 2
                                xt = xsp.tile([P, 4, 512], F32R,
                                              tag=f"xb{j}", name=f"xb{j}")
                                nc.sync.dma_start(
                                    xt[:],
                                    dr["xT"][hf * 512:(hf + 1) * 512,
                                             blk * 512:(blk + 1) * 512]
                                    .rearrange("(k p) t -> p k t", p=P))
                                xbig.append(xt)

                            def x_get(blk, kc):
                                return xbig[blk * 2 + kc // 4][:, kc % 4, :]

                            layer_norm(2, x_get, h1_dst, lnrow["ln1"],
                                       lnp["ln1_b"], ps_ln)
                        proj_fm8(wsp, "sa_wq", bias["sa_bq"], [h1_blocks[0]],
                                 QT, psa, 1.0 / 256.0)
                        proj_fm8(wsp, "sa_wk", bias["sa_bk"], h1_blocks,
                                 KT, psa, 1.0 / 32.0)
                        proj_tm_vaug8(wsp, "sa_wv", h1_blocks, vaug, psa,
                                      1.0 / 32.0)

                # residual base (own half of x)
                for kc in range(NKC):
                    nc.sync.dma_start(
                        xcur[kc][:],
                        dr["xT"][kc * P:(kc + 1) * P, 0:OWN].bitcast(F32))

                with ExitStack() as mscope:
                    mp = mscope.enter_context(
                        tc.tile_pool(name=f"maskp{_rep}", bufs=1))
                    nmt = 4 if mask_mode == "split" else NSC
                    mask_tiles = []
                    for i in range(nmt):
                        mt = mp.tile([P, OWN], F32, tag=f"mask{i}",
                                     name=f"mask{i}")
                        nc.sync.dma_start(mt[:],
                                          dr["maskT"][i * P:(i + 1) * P, :])
                        mask_tiles.append(mt[:])
                    attention_block(
                        "sa", f"sa{_rep}", KT, QT, vaug, mask_tiles,
                        tailb if mask_mode == "split" else None,
                        False, mscope, causal=(mask_mode == "split"))

            # ==================================================================
            # phase 3: cross attention
            # ==================================================================
            # prefetch FFN1's first w1 group now, on the gpsimd DMA queue, so
            # it lands during the CA phase instead of stalling FFN1 startup
            _caff = ExitStack()
            w1pp = _caff.enter_context(
                tc.tile_pool(name=f"w1pre{_rep}", bufs=1))
            w1pre = []
            for kc in range(3):
                wpt = w1pp.tile([P, 1024], F32R, tag=f"wp{kc}",
                                name=f"wp{kc}")
                nc.gpsimd.dma_start(wpt[:], dr["w1"][kc][:, 0:1024])
                w1pre.append(wpt)
            with ExitStack() as ca:
                big = ca.enter_context(tc.tile_pool(name=f"ca_big{_rep}", bufs=1))
                QT = [big.tile([P, OWN], F32R, tag=f"cQT{c}", name=f"cQT{c}")[:]
                      for c in range(NKC)]
                KT = [big.tile([P, T], F32R, tag=f"cKT{c}", name=f"cKT{c}")[:]
                      for c in range(NKC)]
                vaug = [big.tile([P, H, DK + 1], F32R, tag=f"cV{i}",
                                 name=f"cV{i}")[:] for i in range(NSC)]
                for i in range(NSC):
                    nc.vector.memset(vaug[i][:, :, DK].bitcast(F32), 1.0)

                # K/V projections first: they depend only on memory, so
                # they keep the PE busy while the LN2 chain runs.
                with ExitStack() as mm_scope:
                    wsp = mm_scope.enter_context(
                        tc.tile_pool(name=f"ca_wkv{_rep}", bufs=9))
                    pca = mm_scope.enter_context(
                        tc.tile_pool(name=f"ps_ckv{_rep}", bufs=2,
                                     space="PSUM"))
                    with ExitStack() as mscope2:
                        memp = mscope2.enter_context(
                            tc.tile_pool(name=f"mem{_rep}", bufs=1))
                        mem_blocks = []
                        for blk in range(2):
                            mb = []
                            for kp in range(NKC // 2):
                                mt = memp.tile([P, 2, 512], F8,
                                               tag=f"m{blk}_{kp}",
                                               name=f"m{blk}_{kp}")
                                nc.sync.dma_start(
                                    mt[:],
                                    dr["memT8"][kp][:, :,
                                                    blk * 512:(blk + 1) * 512])
                                mb.append(mt[:])
                            mem_blocks.append(mb)

                        with ExitStack() as ph:
                            h2p = ph.enter_context(
                                tc.tile_pool(name=f"h2{_rep}", bufs=1))
                            h2 = [h2p.tile([P, 2, OWN], F8, tag=f"h2_{kp}",
                                           name=f"h2_{kp}")[:]
                                  for kp in range(NKC // 2)]
                            h2_dst = [h2[kc // 2][:, kc % 2, :]
                                      for kc in range(NKC)]

                            def ca_kv_mid():
                                proj_fm8(wsp, "ca_wk", bias["ca_bk"],
                                         mem_blocks, KT, pca, 1.0 / 32.0)
                                proj_tm_vaug8(wsp, "ca_wv", mem_blocks,
                                              vaug, pca, 1.0 / 32.0)

                            with tc.tile_pool(name=f"ps_ln2{_rep}", bufs=1,
                                              space="PSUM") as psl:
                                layer_norm(1, lambda blk, kc: xcur[kc][:],
                                           [h2_dst], lnrow["ln2"],
                                           lnp["ln2_b"], psl, mid=ca_kv_mid)
                            proj_fm8(wsp, "ca_wq", bias["ca_bq"], [h2], QT,
                                     pca, 1.0 / 256.0)

                with ExitStack() as ascope:
                    attention_block("ca", f"ca{_rep}", KT, QT, vaug, None,
                                None, True, ascope)

            # ==================================================================
            # phase 4: FFN
            # ==================================================================
            with ExitStack() as ff:
                ap_pool = ff.enter_context(tc.tile_pool(name=f"aT{_rep}", bufs=1))
                w2p = ff.enter_context(tc.tile_pool(name=f"w2p{_rep}", bufs=2))
                aT = [ap_pool.tile([P, OWN], F32R, tag=f"aT{i}", name=f"aT{i}")[:]
                      for i in range(NFC)]
                with ExitStack() as ph:
                    h3p = ph.enter_context(
                        tc.tile_pool(name=f"h3{_rep}", bufs=1))
                    h3 = [h3p.tile([P, OWN], F32R, tag=f"h3_{kc}",
                                   name=f"h3_{kc}")[:] for kc in range(NKC)]
                    with ExitStack() as wscope:
                        wsp = wscope.enter_context(
                            tc.tile_pool(name=f"ff_ws{_rep}", bufs=10))
                        with tc.tile_pool(name=f"ps_ln3{_rep}", bufs=1,
                                          space="PSUM") as psl:
                            layer_norm(1, lambda blk, kc: xcur[kc][:], [h3],
                                       lnrow["ln3"], lnp["ln3_b"], psl)
                        ps_f1 = wscope.enter_context(
                            tc.tile_pool(name=f"ps_ff1{_rep}", bufs=1,
                                         space="PSUM"))
                        for cg in range(4):
                            if cg == 0:
                                w1t = list(w1pre)
                                for kc in range(3, NKC):
                                    wt = wsp.tile([P, 1024], F32R, tag="w",
                                                  name="w")
                                    nc.sync.dma_start(
                                        wt[:], dr["w1"][kc][:, 0:1024])
                                    w1t.append(wt)
                            else:
                                w1t = []
                                for kc in range(NKC):
                                    wt = wsp.tile([P, 1024], F32R, tag="w",
                                                  name="w")
                                    nc.sync.dma_start(
                                        wt[:], dr["w1"][kc][:, cg * 1024:
                                                            (cg + 1) * 1024])
                                    w1t.append(wt)
                            for cc in range(NKC):
                                cidx = cg * 8 + cc
                                ps = ps_f1.tile([P, 512], F32, tag="proj0",
                                                name="proj0", bufs=4)
                                for kc in range(NKC):
                                    nc.tensor.matmul(
                                        ps[:], w1t[kc][:, cc * P:(cc + 1) * P],
                                        h3[kc], start=(kc == 0),
                                        stop=(kc == NKC - 1))
                                nc.scalar.activation(
                                    aT[cidx], ps[:], AF.Relu,
                                    bias=bias["b1"][:, cidx:cidx + 1])

                with ExitStack() as yscope:
                    ps_y = yscope.enter_context(
                        tc.tile_pool(name=f"ps_y{_rep}", bufs=1, space="PSUM"))
                    yps = [ps_y.tile([P, 512], F32, tag=f"y{c2}", name=f"y{c2}")
                           for c2 in range(NKC)]
                    for kc2 in range(NFC):
                        wt = w2p.tile([P, D], F32R, tag="w", name="w",
                                      bufs=6)
                        nc.sync.dma_start(wt[:], dr["w2"][kc2])
                        for c2 in range(NKC):
                            nc.tensor.matmul(
                                yps[c2][:], wt[:, c2 * P:(c2 + 1) * P],
                                aT[kc2], start=(kc2 == 0),
                                stop=(kc2 == NFC - 1))
                    for c2 in range(NKC):
                        nc.vector.scalar_tensor_tensor(
                            xcur[c2][:], yps[c2][:], bias["b2"][:, c2:c2 + 1],
                            xcur[c2][:], op0=ALU.add, op1=ALU.add)

            _caff.close()

            for c2 in range(NKC):
                nc.sync.dma_start(outT[c2 * P:(c2 + 1) * P, :], xcur[c2][:])

    nc.finalize()
    return nc


# ---------------------------------------------------------------------------
# host side
# ---------------------------------------------------------------------------

def _tile4(wT):
    """[D_in, D_out] -> [D_in/P, D_out/P, P, P] contiguous tiles."""
    di, do = wT.shape
    return np.ascontiguousarray(
        wT.reshape(di // P, P, do // P, P).transpose(0, 2, 1, 3))


def host_prep(inputs):
    f32 = np.float32
    x = np.asarray(inputs["x"], f32)
    mem = np.asarray(inputs["memory"], f32)
    tgt = np.asarray(inputs["tgt_mask"])
    src = np.asarray(inputs["src_mask"])

    add_tgt = (tgt.astype(f32) - 1.0) * 1e9     # [B, T, T]: 0 or -1e9
    add_src = (src.astype(f32) - 1.0) * 1e9     # [B, T]

    import ml_dtypes
    f8 = ml_dtypes.float8_e4m3

    def pack8(wT):
        # [D, cols] -> fp8 x32 k-pair tiles [NKC//2, P, 2, cols]
        d, cols = wT.shape
        return np.ascontiguousarray(
            (wT * 32.0).reshape(d // P // 2, 2, P, cols)
            .transpose(0, 2, 1, 3)).astype(f8)

    shared = {"ones_r": np.ones((P, P), f32)}
    sm_cols = []
    for pre in ("sa", "ca"):
        wq = np.asarray(inputs[f"{pre}_wq"], f32)
        shared[f"{pre}_wq"] = pack8(wq.T)
        shared[f"{pre}_wk"] = pack8(np.asarray(inputs[f"{pre}_wk"], f32).T)
        shared[f"{pre}_wv"] = pack8(np.asarray(inputs[f"{pre}_wv"], f32).T)
        shared[f"{pre}_wo"] = np.ascontiguousarray(
            np.asarray(inputs[f"{pre}_wo"], f32).T.reshape(NKC, P, D))
        shared[f"{pre}_bv"] = np.asarray(inputs[f"{pre}_bv"], f32)
    for pre in ("sa", "ca"):
        sm_cols.append((np.asarray(inputs[f"{pre}_bq"], f32)
                        * 0.125).reshape(NKC, P).T)
        sm_cols.append(np.asarray(inputs[f"{pre}_bk"],
                                  f32).reshape(NKC, P).T)
        sm_cols.append(np.asarray(inputs[f"{pre}_bo"],
                                  f32).reshape(NKC, P).T)
    shared["lnwrows"] = np.concatenate(
        [np.asarray(inputs[f"{ln}_w"], f32) for ln in
         ("ln1", "ln2", "ln3")]).reshape(1, 3 * D)
    shared["w1"] = np.ascontiguousarray(
        np.asarray(inputs["ff_w1"], f32).T.reshape(NKC, P, DFF))
    shared["w2"] = np.ascontiguousarray(
        np.asarray(inputs["ff_w2"], f32).T.reshape(NFC, P, D))
    # order must match sa_bq/sa_bk/sa_bo, ca_bq/ca_bk/ca_bo above
    sm_fixed = [sm_cols[0], sm_cols[1], sm_cols[2],
                sm_cols[3], sm_cols[4], sm_cols[5],
                np.asarray(inputs["ff_b1"], f32).reshape(NFC, P).T,
                np.asarray(inputs["ff_b2"], f32).reshape(NKC, P).T]
    for ln in ("ln1", "ln2", "ln3"):
        sm_fixed.append(np.asarray(inputs[f"{ln}_b"],
                                   f32).reshape(NKC, P).T)


    maps = []
    for c in range(8):
        b, half = c // 2, c % 2
        q0 = half * OWN
        order = np.concatenate(
            [np.arange(q0, q0 + OWN), np.r_[0:q0, q0 + OWN:T]]).astype(
                np.int64)
        m = dict(shared)
        m["xT"] = np.ascontiguousarray(x[b][order].T)
        m["memT"] = np.ascontiguousarray(mem[b].T)
        mT = mem[b].T  # [D, T]
        m["memT8"] = np.ascontiguousarray(
            mT.reshape(NKC // 2, 2, P, T).transpose(0, 2, 1, 3)).astype(f8)
        mt = np.ascontiguousarray(add_tgt[b][q0:q0 + OWN][:, order].T)
        m["maskT"] = mt
        smask_c = add_src[b].reshape(NSC, P).T
        tailb_c = mt[OWN:, 0].reshape(4, P).T
        m["smalls"] = np.ascontiguousarray(
            np.concatenate(sm_fixed + [smask_c, tailb_c], axis=1))
        maps.append(m)
    return maps


def _tail_rows_constant(maps):
    """True when every core's mask s-chunks 4..7 are constant per s-row, so
    they can be applied as a per-partition exp bias instead of tensor adds."""
    for m in maps:
        tail = m["maskT"][OWN:, :]
        if not np.all(tail == tail[:, :1]):
            return False
    return True


def gather(results):
    out = np.zeros((B, T, D), np.float32)
    for c in range(8):
        b, half = c // 2, c % 2
        out[b, half * OWN:(half + 1) * OWN, :] = results[c]["outT"].T
    return out


_NC_CACHE = {}


def kernel(**inputs):
    in_maps = host_prep(inputs)
    mode = "split" if _tail_rows_constant(in_maps) else "full"
    if mode not in _NC_CACHE:
        _NC_CACHE[mode] = build_program(mask_mode=mode)
    nc = _NC_CACHE[mode]
    res = run_bass_kernel_spmd(nc, in_maps, list(range(8)))
    return gather(res.results)


if __name__ == "__main__":
    import reference as ref_mod
    inputs = {k: np.asarray(v) for k, v in ref_mod.setup_inputs().items()}
    expected = np.asarray(ref_mod.reference(**ref_mod.setup_inputs()))
    actual = kernel(**inputs)
    err = np.abs(actual - expected).max()
    rel = err / np.abs(expected).max()
    print("max abs err:", err, "rel:", rel)



# revision 34
# speedup vs baseline: 2.3637x; 1.0043x over previous
"""Transformer decoder layer (pre-norm, self-attn + cross-attn + FFN) on 8
Trainium2 NeuronCores.

Sharding: core c handles batch b = c//2 and the contiguous half of the 1024
target tokens h = c%2 (512 query rows each). K/V work is duplicated within
each batch pair so there are no collectives; every core runs an identical
program on different data. The host rotates the token order per core so that
each core's own tokens are always columns [0, 512) -> one uniform SPMD
program.

On-device layout is feature-major ([d_model, token]) throughout, so no
on-device transposes are needed: the host pre-transposes x / memory / weights
and transposes the output back. Masks are converted to additive fp32 masks on
the host.

Precision/speed: the Q/K/V projections of both attentions run as fp8e4m3
DoubleRow matmuls (x32-scaled weights, contraction k-chunk pairs packed on
tile dim 1; 2 rows/PE-cycle = 4x the f32r rate), with the scale folded into
the f32 PSUM epilogue.  LN outputs (h1/h2) and memory are quantized to fp8
for those projections only; scores, attention-value, output projections, FFN
and the residual stream stay float32r/f32 (measured end-to-end max-rel error
1.05e-2 vs the fp32 reference, tolerance 2e-2).  The attention softmax is
software-pipelined: scores for step i+1 issue on the PE before the AV
matmuls of step i so the DVE/Act exp latency stays off the critical path;
exp feeds an augmented-V matmul whose ones-column accumulates the softmax
denominator, applied via a PE rank-1 broadcast of the reciprocal.  The
cross-attention K/V projections issue between LN2's sum and normalize passes
to hide the LN stats chain.
"""

import numpy as np
from contextlib import ExitStack

import concourse.bass as bass
import concourse.bacc as bacc
import concourse.tile as tile
from concourse import mybir
from concourse.bass_utils import run_bass_kernel_spmd

D = 1024        # d_model
H = 16          # heads
DK = 64         # head dim
DFF = 4096
B = 4
T = 1024        # tgt/src len
OWN = 512       # query rows per core
P = 128         # partitions
NKC = D // P    # 8 feature chunks
NSC = T // P    # 8 s-chunks
NFC = DFF // P  # 32 ffn chunks
EPS = 1e-6

F32 = mybir.dt.float32
F32R = mybir.dt.float32r
F8 = mybir.dt.float8e4
DR = mybir.MatmulPerfMode.DoubleRow
AF = mybir.ActivationFunctionType
ALU = mybir.AluOpType


def _r(ap):
    return ap.bitcast(F32R)


# ---------------------------------------------------------------------------
# program builder (identical for every core; only DRAM contents differ)
# ---------------------------------------------------------------------------

def build_program(repeat=1, mask_mode="split"):
    nc = bacc.Bacc(None)
    dr = {}

    def din(name, shape, dt=F32):
        dr[name] = nc.dram_tensor(name, list(shape), dt, kind="ExternalInput")
        return dr[name]

    din("ones_r", [P, P], F32R)
    din("xT", [D, T], F32R)                # batch-b x, transposed, own tokens first
    din("memT", [D, T], F32R)              # memory[b] transposed
    din("maskT", [T, OWN])           # additive tgt mask, [s_rot, q_local]
    din("lnwrows", [1, 3 * D], F32R)  # LN w rows, concat (PE bcast)
    # all small per-partition tensors packed into one DMA:
    # cols: [sa_bq 8][sa_bk 8][sa_bo 8][ca_bq 8][ca_bk 8][ca_bo 8]
    #       [b1 32][b2 8][ln1_b 8][ln2_b 8][ln3_b 8][smask 8][tailb 4]
    din("smalls", [P, 124])
    for pre in ("sa", "ca"):
        din(f"{pre}_wq", [NKC // 2, P, 2, D], F8)   # x32 scaled fp8, k-pairs
        din(f"{pre}_wk", [NKC // 2, P, 2, D], F8)
        din(f"{pre}_wv", [NKC // 2, P, 2, D], F8)
        din(f"{pre}_wo", [NKC, P, D], F32R)
        din(f"{pre}_bv", [D], F32R)      # placed at partition 64 (outer trick)
    din("memT8", [NKC // 2, P, 2, T], F8)  # fp8 memory, k-pairs
    din("w1", [NKC, P, DFF], F32R)
    din("w2", [NFC, P, D], F32R)

    outT = nc.dram_tensor("outT", [D, OWN], F32, kind="ExternalOutput")

    with ExitStack() as ctx:
        tc = ctx.enter_context(tile.TileContext(nc))
        ctx.enter_context(nc.allow_low_precision(
            reason="float32r staging for full-rate PE matmuls"))
        persist = ctx.enter_context(tc.tile_pool(name="persist", bufs=1))

        ones = persist.tile([P, P], F32R, tag="ones", name="ones")
        nc.gpsimd.dma_start(ones[:], dr["ones_r"][:])
        ones_f = persist.tile([P, 1], F32, tag="ones_f", name="ones_f")
        nc.vector.memset(ones_f[:], 1.0)
        eps_11 = persist.tile([1, 1], F32, tag="eps11", name="eps11")
        nc.vector.memset(eps_11[:], EPS)

        smalls = persist.tile([P, 124], F32, tag="smalls", name="smalls")
        nc.gpsimd.dma_start(smalls[:], dr["smalls"][:])
        _off = [0]

        def s_col(n):
            t = smalls[:, _off[0]:_off[0] + n]
            _off[0] += n
            return t

        bias = {}
        for pre in ("sa", "ca"):
            for nm in ("bq", "bk", "bo"):
                bias[f"{pre}_{nm}"] = s_col(NKC)
        bias["b1"] = s_col(NFC)
        bias["b2"] = s_col(NKC)
        lnp = {}
        for ln in ("ln1", "ln2", "ln3"):
            lnp[f"{ln}_b"] = s_col(NKC)
        smask = s_col(NSC)
        tailb = s_col(4)
        for pre in ("sa", "ca"):
            bv = persist.tile([P, D], F32R, tag=f"{pre}_bv", name=f"{pre}_bv")
            nc.gpsimd.dma_start(bv[64:65, :], dr[f"{pre}_bv"][None, :])
            bias[f"{pre}_bv"] = bv
        lnwr = persist.tile([1, 3 * D], F32R, tag="lnwrows", name="lnwrows")
        nc.gpsimd.dma_start(lnwr[:], dr["lnwrows"][:])
        lnrow = {"ln1": lnwr[0:1, 0:D], "ln2": lnwr[0:1, D:2 * D],
                 "ln3": lnwr[0:1, 2 * D:3 * D]}

        # residual (own tokens), lives to the end
        xcur = [persist.tile([P, OWN], F32, tag=f"xc{i}", name=f"xc{i}")
                for i in range(NKC)]

        tmp = ctx.enter_context(tc.tile_pool(name="tmp", bufs=2))
        stats = ctx.enter_context(tc.tile_pool(name="stats", bufs=1))

        # ------------------------------------------------------------------
        def ln_sums(blk, src_get, ps_st, nbuf=1):
            """Pass 1: PE ones-reductions for sum and sum-of-squares."""
            sb0 = src_get(blk, 0)
            src_is_r = sb0.dtype == F32R
            ones_s = ones[:, 0:1] if src_is_r else ones_f[:]
            ps_s = ps_st.tile([1, 512], F32, tag="lns", name="lns", bufs=nbuf)
            ps_q = ps_st.tile([1, 512], F32, tag="lnq", name="lnq", bufs=nbuf)
            for kc in range(NKC):
                sbk = sb0 if kc == 0 else src_get(blk, kc)
                nc.tensor.matmul(ps_s[:], ones_s, sbk,
                                 start=(kc == 0), stop=(kc == NKC - 1))
                sq = tmp.tile([P, 512], F32R, tag="lnsq", name="lnsq", bufs=1)
                nc.scalar.activation(sq[:], sbk, AF.Square)
                nc.tensor.matmul(ps_q[:], ones[:, 0:1], sq[:],
                                 start=(kc == 0), stop=(kc == NKC - 1))
            return ps_s, ps_q

        def ln_normalize(blk, src_get, db, wrow, b_pn, ps_st, ps_sq):
            """Pass 2: stats chain (DVE/Act) + PE broadcasts + apply."""
            ps_s, ps_q = ps_sq
            s2 = stats.tile([1, 512], F32, tag="lnstA", name="lnstA")
            # s2 = (sum/sqrt(D*(D-1)))^2 = sum^2/(D*(D-1))
            nc.scalar.activation(s2[:], ps_s[:], AF.Square,
                                 scale=float(1.0 / np.sqrt(D * (D - 1.0))))
            var = stats.tile([1, 512], F32, tag="lnstB", name="lnstB")
            nc.vector.scalar_tensor_tensor(
                var[:], ps_q[:], 1.0 / (D - 1.0), s2[:],
                op0=ALU.mult, op1=ALU.subtract)
            std = stats.tile([1, 512], F32, tag="lnstA", name="lnstA")
            nc.scalar.activation(std[:], var[:], AF.Sqrt)
            nc.scalar.add(std[:], std[:], eps_11[:])
            rstd = stats.tile([1, 512], F32R, tag="lnstC", name="lnstC")
            nc.vector.reciprocal(rstd[:], std[:])
            mr = stats.tile([1, 512], F32R, tag="lnstB", name="lnstB")
            nc.vector.scalar_tensor_tensor(
                mr[:], ps_s[:], 1.0 / D, rstd[:],
                op0=ALU.mult, op1=ALU.mult)
            for kc in range(NKC):
                wl = wrow[0:1, kc * P:(kc + 1) * P]
                ps_rb = ps_st.tile([P, 512], F32, tag="ln_rb",
                                   name="ln_rb", bufs=1)
                nc.tensor.matmul(ps_rb[:], wl, rstd[:],
                                 start=True, stop=True)
                ps_mb = ps_st.tile([P, 512], F32, tag="ln_mb",
                                   name="ln_mb", bufs=1)
                nc.tensor.matmul(ps_mb[:], wl, mr[:],
                                 start=True, stop=True)
                t = tmp.tile([P, 512], F32, tag="lnt", name="lnt")
                nc.vector.tensor_mul(t[:], src_get(blk, kc), ps_rb[:])
                nc.vector.scalar_tensor_tensor(
                    db[kc], t[:], b_pn[:, kc:kc + 1], ps_mb[:],
                    op0=ALU.add, op1=ALU.subtract)

        def layer_norm(nblk, src_get, dst_blocks, wrow, b_pn, ps_st,
                       mid=None):
            """Feature-major LN, h = (x - mean) * (w * rstd) + b.
            All blocks' sum passes issue first (so block b+1's PE sums hide
            block b's DVE stats chain), then `mid()` may issue extra PE work,
            then the normalize passes run."""
            if mid is None:
                for blk in range(nblk):
                    s = ln_sums(blk, src_get, ps_st)
                    ln_normalize(blk, src_get, dst_blocks[blk], wrow, b_pn,
                                 ps_st, s)
            else:
                sums = [ln_sums(blk, src_get, ps_st, nbuf=nblk)
                        for blk in range(nblk)]
                mid()
                for blk in range(nblk):
                    ln_normalize(blk, src_get, dst_blocks[blk], wrow, b_pn,
                                 ps_st, sums[blk])

        # ------------------------------------------------------------------
        def load_w_rows(wpool, wname, n=NKC, cols=D):
            tiles = []
            for kc in range(n):
                wt = wpool.tile([P, cols], F32R, tag="w", name="w")
                nc.sync.dma_start(wt[:], dr[wname][kc])
                tiles.append(wt)
            return tiles

        def load_w_pairs(wpool, wname, n=NKC // 2, cols=D):
            """fp8 weight k-pair tiles [P, 2, cols] for DoubleRow."""
            tiles = []
            for kp in range(n):
                wt = wpool.tile([P, 2, cols], F8, tag="w8", name="w8")
                nc.sync.dma_start(wt[:], dr[wname][kp])
                tiles.append(wt)
            return tiles

        def proj_fm8(wpool, wname, bias_pn, src_blocks, dst, ps_acc, scl):
            """fp8 DoubleRow feature-major projection.
            dst[c][:, blk*512:..] = scl * sum_kp W8[kp,c].T @ src8[blk][kp] + b
            src8 blocks are lists of [P, 2, 512] fp8 pair tiles."""
            nblk = len(src_blocks)
            w_tiles = load_w_pairs(wpool, wname)
            for c in range(NKC):
                pss = [ps_acc.tile([P, 512], F32, tag=f"proj{blk}",
                                   name=f"proj{blk}") for blk in range(nblk)]
                for kp in range(NKC // 2):
                    for blk in range(nblk):
                        nc.tensor.matmul(pss[blk][:],
                                         w_tiles[kp][:, :, c * P:(c + 1) * P],
                                         src_blocks[blk][kp][:],
                                         start=(kp == 0),
                                         stop=(kp == NKC // 2 - 1),
                                         perf_mode=DR)
                for blk in range(nblk):
                    nc.vector.tensor_scalar(
                        dst[c][:, blk * 512:(blk + 1) * 512],
                        pss[blk][:], scl, bias_pn[:, c:c + 1],
                        op0=ALU.mult, op1=ALU.add)

        def proj_tm_vaug8(wpool, wname, src_blocks, vaug, ps_acc, scl):
            """fp8 DoubleRow token-major V projection into vaug tiles."""
            wv = load_w_pairs(wpool, wname)
            for dc in range(2):
                for st in range(NSC):
                    sb = src_blocks[st // 4]
                    t0 = (st % 4) * P
                    ps = ps_acc.tile([P, 512], F32, tag="proj0", name="proj0")
                    for kp in range(NKC // 2):
                        nc.tensor.matmul(
                            ps[:], sb[kp][:, :, t0:t0 + P],
                            wv[kp][:, :, dc * 512:(dc + 1) * 512],
                            start=(kp == 0), stop=(kp == NKC // 2 - 1),
                            perf_mode=DR)
                    nc.vector.tensor_scalar_mul(
                        vaug[st][:, 8 * dc:8 * (dc + 1), 0:DK],
                        ps[:].rearrange("p (h d) -> p h d", h=8), scl)

        # ------------------------------------------------------------------
        def attention(KT, QT, vaug, OT, mask_tiles, tail_pn, smask_pn,
                      bv_tile, att_pools, causal=False):
            """Software-pipelined: scores for step s+1 issue on the PE before
            the AV matmuls of step s, so the DVE/Act exp latency of step s
            hides under PE work.  PSUM: 4 score banks + 2 po + 1 rb = 7.
            causal=True: own-block s-chunk i only has unmasked queries
            q >= 128*i (host rotation puts own tokens first, ascending), so
            scores/mask-add/exp/AV shrink to that q-range -- exact, since the
            skipped exp values are 0 and i=0 initializes the full PSUM."""
            ps_sc, ps_av, ps_rb, epool = att_pools
            nmask = len(mask_tiles) if mask_tiles is not None else 0

            def q0_of(i):
                return i * P if (causal and i < nmask) else 0

            def issue_scores(c, i):
                q0 = q0_of(i)
                pair = [ps_sc.tile([P, 512], F32, tag="sc", name="sc", bufs=4),
                        ps_sc.tile([P, 512], F32, tag="sc", name="sc", bufs=4)]
                for h01 in (0, 1):
                    sl = slice(64 * h01, 64 * h01 + 64)
                    nc.tensor.matmul(
                        pair[h01][:, q0:512], KT[c][sl, i * P:(i + 1) * P],
                        QT[c][sl, q0:512], start=True, stop=True)
                return pair

            def issue_exp(i, pssc_h):
                q0 = q0_of(i)
                e = epool.tile([P, 512], F32R, tag="e", name="e")
                if i < nmask:
                    tm = epool.tile([P, 512], F32, tag="emask",
                                    name="emask", bufs=2)
                    nc.vector.tensor_add(tm[:, q0:512], pssc_h[:, q0:512],
                                         mask_tiles[i][:, q0:512])
                    nc.scalar.activation(e[:, q0:512], tm[:, q0:512], AF.Exp)
                elif tail_pn is not None:
                    nc.scalar.activation(e[:], pssc_h[:], AF.Exp,
                                         bias=tail_pn[:, i - 4:i - 3])
                elif smask_pn is not None:
                    nc.scalar.activation(e[:], pssc_h[:], AF.Exp,
                                         bias=smask_pn[:, i:i + 1])
                else:
                    nc.scalar.activation(e[:], pssc_h[:], AF.Exp)
                return e

            seq = [(c, i) for c in range(NKC) for i in range(NSC)]
            po_c = None
            pair = issue_scores(*seq[0])
            for idx, (c, i) in enumerate(seq):
                q0 = q0_of(i)
                if i == 0:
                    po_c = [ps_av.tile([DK + 1, 512], F32, tag="po",
                                       name="po", bufs=3),
                            ps_av.tile([DK + 1, 512], F32, tag="po",
                                       name="po", bufs=3)]
                es = [issue_exp(i, pair[0]), issue_exp(i, pair[1])]
                if idx + 1 < len(seq):
                    pair = issue_scores(*seq[idx + 1])
                for h01 in (0, 1):
                    nc.tensor.matmul(
                        po_c[h01][:, q0:512], vaug[i][:, 2 * c + h01, :],
                        es[h01][:, q0:512], start=(i == 0),
                        stop=(i == NSC - 1), skip_group_check=True)
                if i == NSC - 1:
                    for h01 in (0, 1):
                        heng = nc.vector
                        h = 2 * c + h01
                        po = po_c
                        sums = epool.tile([P, 512], F32R, tag="sums",
                                          name="sums", bufs=2)
                        heng.tensor_copy(sums[64:65, :],
                                         po[h01][DK:DK + 1, :])
                        # O_un += bv (x) sums (folds V bias through softmax)
                        nc.tensor.matmul(
                            po[h01][0:DK, :],
                            bv_tile[64:65, DK * h:DK * h + DK],
                            sums[64:65, :], start=False, stop=True,
                            skip_group_check=True)
                        nc.vector.reciprocal(sums[64:65, :], sums[64:65, :])
                        prb = ps_rb.tile([DK, 512], F32, tag="rb", name="rb")
                        nc.tensor.matmul(prb[:], ones[64:65, 0:DK],
                                         sums[64:65, :], start=True, stop=True)
                        # DVE can read only one PSUM operand: stage po first
                        o_un = epool.tile([DK, 512], F32, tag="o_un",
                                          name="o_un", bufs=2)
                        heng.tensor_copy(o_un[:], po[h01][0:DK, :])
                        if h01 == 0:
                            heng.tensor_mul(OT[c][0:DK, :], o_un[:],
                                            prb[:])
                        else:
                            ot = epool.tile([DK, 512], F32R, tag="ot",
                                            name="ot", bufs=1)
                            nc.vector.tensor_mul(ot[:], o_un[:], prb[:])
                            nc.sync.dma_start(OT[c][DK:P, :], ot[:])

        def out_proj_residual(w_tiles, bias_pn, OT, ps_acc):
            for c2 in range(NKC):
                ps = ps_acc.tile([P, 512], F32, tag="proj0", name="proj0")
                for c in range(NKC):
                    nc.tensor.matmul(ps[:],
                                     w_tiles[c][:, c2 * P:(c2 + 1) * P],
                                     OT[c][:],
                                     start=(c == 0), stop=(c == NKC - 1))
                nc.vector.scalar_tensor_tensor(
                    xcur[c2][:], ps[:], bias_pn[:, c2:c2 + 1], xcur[c2][:],
                    op0=ALU.add, op1=ALU.add)

        def attention_block(prefix, pn, KT, QT, vaug, mask_tiles, tail_pn,
                            use_smask, scope, causal=False):
            """Runs attention + output projection + residual."""
            otp = scope.enter_context(
                tc.tile_pool(name=f"{pn}_otp", bufs=1))
            OT = [otp.tile([P, OWN], F32R, tag=f"OT{c}", name=f"OT{c}")[:]
                  for c in range(NKC)]
            # wo prefetch overlaps the attention phase
            wsp = scope.enter_context(tc.tile_pool(name=f"{pn}_wso", bufs=8))
            wo_tiles = load_w_rows(wsp, f"{prefix}_wo")
            with ExitStack() as att:
                ps_sc = att.enter_context(
                    tc.tile_pool(name=f"{pn}_psc", bufs=4, space="PSUM"))
                ps_av = att.enter_context(
                    tc.tile_pool(name=f"{pn}_pav", bufs=2, space="PSUM"))
                ps_rb = att.enter_context(
                    tc.tile_pool(name=f"{pn}_prb", bufs=1, space="PSUM"))
                epool = att.enter_context(
                    tc.tile_pool(name=f"{pn}_ep", bufs=3))
                attention(KT, QT, vaug, OT, mask_tiles, tail_pn,
                          smask if use_smask else None,
                          bias[f"{prefix}_bv"], (ps_sc, ps_av, ps_rb, epool),
                          causal=causal)
            with tc.tile_pool(name=f"{pn}_pso", bufs=4, space="PSUM") as pso:
                out_proj_residual(wo_tiles, bias[f"{prefix}_bo"], OT, pso)

        for _rep in range(repeat):
            # ==================================================================
            # phase 1+2: LN1, self-attention
            # ==================================================================
            with ExitStack() as sa:
                big = sa.enter_context(tc.tile_pool(name=f"sa_big{_rep}", bufs=1))
                QT = [big.tile([P, OWN], F32R, tag=f"QT{c}", name=f"QT{c}")[:]
                      for c in range(NKC)]
                KT = [big.tile([P, T], F32R, tag=f"KT{c}", name=f"KT{c}")[:]
                      for c in range(NKC)]
                vaug = [big.tile([P, H, DK + 1], F32R, tag=f"V{i}", name=f"V{i}")[:]
                        for i in range(NSC)]
                for i in range(NSC):
                    nc.vector.memset(vaug[i][:, :, DK].bitcast(F32), 1.0)

                with ExitStack() as ph:
                    h1p = ph.enter_context(
                        tc.tile_pool(name=f"h1{_rep}", bufs=1))
                    h1_blocks = [
                        [h1p.tile([P, 2, 512], F8, tag=f"h1_{blk}_{kp}",
                                  name=f"h1_{blk}_{kp}")[:]
                         for kp in range(NKC // 2)]
                        for blk in range(2)]
                    h1_dst = [
                        [h1_blocks[blk][kc // 2][:, kc % 2, :]
                         for kc in range(NKC)]
                        for blk in range(2)]
                    with ExitStack() as wscope:
                        # weight pool open during LN1 so the Q/K/V weight
                        # prefetch overlaps the LN chain
                        wsp = wscope.enter_context(
                            tc.tile_pool(name=f"sa_ws{_rep}", bufs=9))
                        psa = wscope.enter_context(
                            tc.tile_pool(name=f"ps_sap{_rep}", bufs=2,
                                         space="PSUM"))
                        with ExitStack() as lnscope:
                            xsp = lnscope.enter_context(
                                tc.tile_pool(name=f"xs{_rep}", bufs=1))
                            ps_ln = lnscope.enter_context(
                                tc.tile_pool(name=f"ps_ln1{_rep}", bufs=1,
                                             space="PSUM"))
                            # x resident as 4 batched tiles: one DMA per
                            # 4 feature chunks (amortizes descriptor cost),
                            # read by both the sum and the apply pass
                            # first chunk alone so the first LN1 matmul
                            # starts after 256KB, not 1MB; rest batched
                            x0 = xsp.tile([P, 1, 512], F32R, tag="xb0",
                                          name="xb0")
                            nc.sync.dma_start(
                                x0[:],
                                dr["xT"][0:P, 0:512]
                                .rearrange("(k p) t -> p k t", p=P))
                            x13 = xsp.tile([P, 3, 512], F32R, tag="xb13",
                                           name="xb13")
                            nc.sync.dma_start(
                                x13[:],
                                dr["xT"][P:4 * P, 0:512]
                                .rearrange("(k p) t -> p k t", p=P))
                            xbig = []
                            for j in range(3):
                                blk, hf = (j + 1) // 2, (j + 1) % 2
                                xt = xsp.tile([P, 4, 512], F32R,
                                              tag=f"xb{j + 1}",
                                              name=f"xb{j + 1}")
                                nc.sync.dma_start(
                                    xt[:],
                                    dr["xT"][hf * 512:(hf + 1) * 512,
                                             blk * 512:(blk + 1) * 512]
                                    .rearrange("(k p) t -> p k t", p=P))
                                xbig.append(xt)

                            def x_get(blk, kc):
                                if blk == 0 and kc == 0:
                                    return x0[:, 0, :]
                                if blk == 0 and kc < 4:
                                    return x13[:, kc - 1, :]
                                return xbig[blk * 2 + kc // 4 - 1][:, kc % 4, :]

                            layer_norm(2, x_get, h1_dst, lnrow["ln1"],
                                       lnp["ln1_b"], ps_ln)
                        proj_fm8(wsp, "sa_wq", bias["sa_bq"], [h1_blocks[0]],
                                 QT, psa, 1.0 / 256.0)
                        proj_fm8(wsp, "sa_wk", bias["sa_bk"], h1_blocks,
                                 KT, psa, 1.0 / 32.0)
                        proj_tm_vaug8(wsp, "sa_wv", h1_blocks, vaug, psa,
                                      1.0 / 32.0)

                # residual base (own half of x)
                for kc in range(NKC):
                    nc.sync.dma_start(
                        xcur[kc][:],
                        dr["xT"][kc * P:(kc + 1) * P, 0:OWN].bitcast(F32))

                with ExitStack() as mscope:
                    mp = mscope.enter_context(
                        tc.tile_pool(name=f"maskp{_rep}", bufs=1))
                    nmt = 4 if mask_mode == "split" else NSC
                    mask_tiles = []
                    for i in range(nmt):
                        mt = mp.tile([P, OWN], F32, tag=f"mask{i}",
                                     name=f"mask{i}")
                        nc.sync.dma_start(mt[:],
                                          dr["maskT"][i * P:(i + 1) * P, :])
                        mask_tiles.append(mt[:])
                    attention_block(
                        "sa", f"sa{_rep}", KT, QT, vaug, mask_tiles,
                        tailb if mask_mode == "split" else None,
                        False, mscope, causal=(mask_mode == "split"))

            # ==================================================================
            # phase 3: cross attention
            # ==================================================================
            # prefetch FFN1's first w1 group now, on the gpsimd DMA queue, so
            # it lands during the CA phase instead of stalling FFN1 startup
            _caff = ExitStack()
            w1pp = _caff.enter_context(
                tc.tile_pool(name=f"w1pre{_rep}", bufs=1))
            w1pre = []
            for kc in range(3):
                wpt = w1pp.tile([P, 1024], F32R, tag=f"wp{kc}",
                                name=f"wp{kc}")
                nc.gpsimd.dma_start(wpt[:], dr["w1"][kc][:, 0:1024])
                w1pre.append(wpt)
            with ExitStack() as ca:
                big = ca.enter_context(tc.tile_pool(name=f"ca_big{_rep}", bufs=1))
                QT = [big.tile([P, OWN], F32R, tag=f"cQT{c}", name=f"cQT{c}")[:]
                      for c in range(NKC)]
                KT = [big.tile([P, T], F32R, tag=f"cKT{c}", name=f"cKT{c}")[:]
                      for c in range(NKC)]
                vaug = [big.tile([P, H, DK + 1], F32R, tag=f"cV{i}",
                                 name=f"cV{i}")[:] for i in range(NSC)]
                for i in range(NSC):
                    nc.vector.memset(vaug[i][:, :, DK].bitcast(F32), 1.0)

                # K/V projections first: they depend only on memory, so
                # they keep the PE busy while the LN2 chain runs.
                with ExitStack() as mm_scope:
                    wsp = mm_scope.enter_context(
                        tc.tile_pool(name=f"ca_wkv{_rep}", bufs=9))
                    pca = mm_scope.enter_context(
                        tc.tile_pool(name=f"ps_ckv{_rep}", bufs=2,
                                     space="PSUM"))
                    with ExitStack() as mscope2:
                        memp = mscope2.enter_context(
                            tc.tile_pool(name=f"mem{_rep}", bufs=1))
                        mem_blocks = []
                        for blk in range(2):
                            mb = []
                            for kp in range(NKC // 2):
                                mt = memp.tile([P, 2, 512], F8,
                                               tag=f"m{blk}_{kp}",
                                               name=f"m{blk}_{kp}")
                                nc.sync.dma_start(
                                    mt[:],
                                    dr["memT8"][kp][:, :,
                                                    blk * 512:(blk + 1) * 512])
                                mb.append(mt[:])
                            mem_blocks.append(mb)

                        with ExitStack() as ph:
                            h2p = ph.enter_context(
                                tc.tile_pool(name=f"h2{_rep}", bufs=1))
                            h2 = [h2p.tile([P, 2, OWN], F8, tag=f"h2_{kp}",
                                           name=f"h2_{kp}")[:]
                                  for kp in range(NKC // 2)]
                            h2_dst = [h2[kc // 2][:, kc % 2, :]
                                      for kc in range(NKC)]

                            def ca_kv_mid():
                                proj_fm8(wsp, "ca_wk", bias["ca_bk"],
                                         mem_blocks, KT, pca, 1.0 / 32.0)
                                proj_tm_vaug8(wsp, "ca_wv", mem_blocks,
                                              vaug, pca, 1.0 / 32.0)

                            with tc.tile_pool(name=f"ps_ln2{_rep}", bufs=1,
                                              space="PSUM") as psl:
                                layer_norm(1, lambda blk, kc: xcur[kc][:],
                                           [h2_dst], lnrow["ln2"],
                                           lnp["ln2_b"], psl, mid=ca_kv_mid)
                            proj_fm8(wsp, "ca_wq", bias["ca_bq"], [h2], QT,
                                     pca, 1.0 / 256.0)

                with ExitStack() as ascope:
                    attention_block("ca", f"ca{_rep}", KT, QT, vaug, None,
                                None, True, ascope)

            # ==================================================================
            # phase 4: FFN
            # ==================================================================
            with ExitStack() as ff:
                ap_pool = ff.enter_context(tc.tile_pool(name=f"aT{_rep}", bufs=1))
                w2p = ff.enter_context(tc.tile_pool(name=f"w2p{_rep}", bufs=2))
                aT = [ap_pool.tile([P, OWN], F32R, tag=f"aT{i}", name=f"aT{i}")[:]
                      for i in range(NFC)]
                with ExitStack() as ph:
                    h3p = ph.enter_context(
                        tc.tile_pool(name=f"h3{_rep}", bufs=1))
                    h3 = [h3p.tile([P, OWN], F32R, tag=f"h3_{kc}",
                                   name=f"h3_{kc}")[:] for kc in range(NKC)]
                    with ExitStack() as wscope:
                        wsp = wscope.enter_context(
                            tc.tile_pool(name=f"ff_ws{_rep}", bufs=10))
                        with tc.tile_pool(name=f"ps_ln3{_rep}", bufs=1,
                                          space="PSUM") as psl:
                            layer_norm(1, lambda blk, kc: xcur[kc][:], [h3],
                                       lnrow["ln3"], lnp["ln3_b"], psl)
                        ps_f1 = wscope.enter_context(
                            tc.tile_pool(name=f"ps_ff1{_rep}", bufs=1,
                                         space="PSUM"))
                        for cg in range(4):
                            if cg == 0:
                                w1t = list(w1pre)
                                for kc in range(3, NKC):
                                    wt = wsp.tile([P, 1024], F32R, tag="w",
                                                  name="w")
                                    nc.sync.dma_start(
                                        wt[:], dr["w1"][kc][:, 0:1024])
                                    w1t.append(wt)
                            else:
                                w1t = []
                                for kc in range(NKC):
                                    wt = wsp.tile([P, 1024], F32R, tag="w",
                                                  name="w")
                                    nc.sync.dma_start(
                                        wt[:], dr["w1"][kc][:, cg * 1024:
                                                            (cg + 1) * 1024])
                                    w1t.append(wt)
                            for cc in range(NKC):
                                cidx = cg * 8 + cc
                                ps = ps_f1.tile([P, 512], F32, tag="proj0",
                                                name="proj0", bufs=4)
                                for kc in range(NKC):
                                    nc.tensor.matmul(
                                        ps[:], w1t[kc][:, cc * P:(cc + 1) * P],
                                        h3[kc], start=(kc == 0),
                                        stop=(kc == NKC - 1))
                                nc.scalar.activation(
                                    aT[cidx], ps[:], AF.Relu,
                                    bias=bias["b1"][:, cidx:cidx + 1])

                with ExitStack() as yscope:
                    ps_y = yscope.enter_context(
                        tc.tile_pool(name=f"ps_y{_rep}", bufs=1, space="PSUM"))
                    yps = [ps_y.tile([P, 512], F32, tag=f"y{c2}", name=f"y{c2}")
                           for c2 in range(NKC)]
                    for kc2 in range(NFC):
                        wt = w2p.tile([P, D], F32R, tag="w", name="w",
                                      bufs=6)
                        nc.sync.dma_start(wt[:], dr["w2"][kc2])
                        for c2 in range(NKC):
                            nc.tensor.matmul(
                                yps[c2][:], wt[:, c2 * P:(c2 + 1) * P],
                                aT[kc2], start=(kc2 == 0),
                                stop=(kc2 == NFC - 1))
                    for c2 in range(NKC):
                        nc.vector.scalar_tensor_tensor(
                            xcur[c2][:], yps[c2][:], bias["b2"][:, c2:c2 + 1],
                            xcur[c2][:], op0=ALU.add, op1=ALU.add)

            _caff.close()

            for c2 in range(NKC):
                nc.sync.dma_start(outT[c2 * P:(c2 + 1) * P, :], xcur[c2][:])

    nc.finalize()
    return nc


# ---------------------------------------------------------------------------
# host side
# ---------------------------------------------------------------------------

def _tile4(wT):
    """[D_in, D_out] -> [D_in/P, D_out/P, P, P] contiguous tiles."""
    di, do = wT.shape
    return np.ascontiguousarray(
        wT.reshape(di // P, P, do // P, P).transpose(0, 2, 1, 3))


def host_prep(inputs):
    f32 = np.float32
    x = np.asarray(inputs["x"], f32)
    mem = np.asarray(inputs["memory"], f32)
    tgt = np.asarray(inputs["tgt_mask"])
    src = np.asarray(inputs["src_mask"])

    add_tgt = (tgt.astype(f32) - 1.0) * 1e9     # [B, T, T]: 0 or -1e9
    add_src = (src.astype(f32) - 1.0) * 1e9     # [B, T]

    import ml_dtypes
    f8 = ml_dtypes.float8_e4m3

    def pack8(wT):
        # [D, cols] -> fp8 x32 k-pair tiles [NKC//2, P, 2, cols]
        d, cols = wT.shape
        return np.ascontiguousarray(
            (wT * 32.0).reshape(d // P // 2, 2, P, cols)
            .transpose(0, 2, 1, 3)).astype(f8)

    shared = {"ones_r": np.ones((P, P), f32)}
    sm_cols = []
    for pre in ("sa", "ca"):
        wq = np.asarray(inputs[f"{pre}_wq"], f32)
        shared[f"{pre}_wq"] = pack8(wq.T)
        shared[f"{pre}_wk"] = pack8(np.asarray(inputs[f"{pre}_wk"], f32).T)
        shared[f"{pre}_wv"] = pack8(np.asarray(inputs[f"{pre}_wv"], f32).T)
        shared[f"{pre}_wo"] = np.ascontiguousarray(
            np.asarray(inputs[f"{pre}_wo"], f32).T.reshape(NKC, P, D))
        shared[f"{pre}_bv"] = np.asarray(inputs[f"{pre}_bv"], f32)
    for pre in ("sa", "ca"):
        sm_cols.append((np.asarray(inputs[f"{pre}_bq"], f32)
                        * 0.125).reshape(NKC, P).T)
        sm_cols.append(np.asarray(inputs[f"{pre}_bk"],
                                  f32).reshape(NKC, P).T)
        sm_cols.append(np.asarray(inputs[f"{pre}_bo"],
                                  f32).reshape(NKC, P).T)
    shared["lnwrows"] = np.concatenate(
        [np.asarray(inputs[f"{ln}_w"], f32) for ln in
         ("ln1", "ln2", "ln3")]).reshape(1, 3 * D)
    shared["w1"] = np.ascontiguousarray(
        np.asarray(inputs["ff_w1"], f32).T.reshape(NKC, P, DFF))
    shared["w2"] = np.ascontiguousarray(
        np.asarray(inputs["ff_w2"], f32).T.reshape(NFC, P, D))
    # order must match sa_bq/sa_bk/sa_bo, ca_bq/ca_bk/ca_bo above
    sm_fixed = [sm_cols[0], sm_cols[1], sm_cols[2],
                sm_cols[3], sm_cols[4], sm_cols[5],
                np.asarray(inputs["ff_b1"], f32).reshape(NFC, P).T,
                np.asarray(inputs["ff_b2"], f32).reshape(NKC, P).T]
    for ln in ("ln1", "ln2", "ln3"):
        sm_fixed.append(np.asarray(inputs[f"{ln}_b"],
                                   f32).reshape(NKC, P).T)


    maps = []
    for c in range(8):
        b, half = c // 2, c % 2
        q0 = half * OWN
        order = np.concatenate(
            [np.arange(q0, q0 + OWN), np.r_[0:q0, q0 + OWN:T]]).astype(
                np.int64)
        m = dict(shared)
        m["xT"] = np.ascontiguousarray(x[b][order].T)
        m["memT"] = np.ascontiguousarray(mem[b].T)
        mT = mem[b].T  # [D, T]
        m["memT8"] = np.ascontiguousarray(
            mT.reshape(NKC // 2, 2, P, T).transpose(0, 2, 1, 3)).astype(f8)
        mt = np.ascontiguousarray(add_tgt[b][q0:q0 + OWN][:, order].T)
        m["maskT"] = mt
        smask_c = add_src[b].reshape(NSC, P).T
        tailb_c = mt[OWN:, 0].reshape(4, P).T
        m["smalls"] = np.ascontiguousarray(
            np.concatenate(sm_fixed + [smask_c, tailb_c], axis=1))
        maps.append(m)
    return maps


def _tail_rows_constant(maps):
    """True when every core's mask s-chunks 4..7 are constant per s-row, so
    they can be applied as a per-partition exp bias instead of tensor adds."""
    for m in maps:
        tail = m["maskT"][OWN:, :]
        if not np.all(tail == tail[:, :1]):
            return False
    return True


def gather(results):
    out = np.zeros((B, T, D), np.float32)
    for c in range(8):
        b, half = c // 2, c % 2
        out[b, half * OWN:(half + 1) * OWN, :] = results[c]["outT"].T
    return out


_NC_CACHE = {}


def kernel(**inputs):
    in_maps = host_prep(inputs)
    mode = "split" if _tail_rows_constant(in_maps) else "full"
    if mode not in _NC_CACHE:
        _NC_CACHE[mode] = build_program(mask_mode=mode)
    nc = _NC_CACHE[mode]
    res = run_bass_kernel_spmd(nc, in_maps, list(range(8)))
    return gather(res.results)


if __name__ == "__main__":
    import reference as ref_mod
    inputs = {k: np.asarray(v) for k, v in ref_mod.setup_inputs().items()}
    expected = np.asarray(ref_mod.reference(**ref_mod.setup_inputs()))
    actual = kernel(**inputs)
    err = np.abs(actual - expected).max()
    rel = err / np.abs(expected).max()
    print("max abs err:", err, "rel:", rel)

